# revision 56
# baseline (speedup 1.0000x reference)
"""Local (banded) attention kernel for Trainium2, sharded over 8 NeuronCores.

Sharding: core c handles batch b=c//4 and heads 4*(c%4)..4*(c%4)+3.
Host pre-transposes x and weight slices; device does QKV projection,
banded attention (window 128 -> only tile-diagonal +/-1 blocks), and the
per-core slice of the output projection. Host sums the 4 partial outputs
per batch and adds the output bias (including the folded V bias: since
softmax rows sum to 1, att@(v+bv) = att@v + bv, so bv@Wp^T moves to the
host-side output bias).

Mixed precision:
- q/k projections: fp8 DoubleRow with weights pre-scaled by 32 (keeps the
  small weights out of fp8's subnormal range); the energy exp scale
  absorbs the 32*32 factor.
- v projection: fp8 DoubleRow with residual compensation at one common
  scale VS=2048: v*VS = x8@wv8 + xr@wv8 + x8@wvr where x8 = fp8(x),
  xr = fp8(x - x8), wv8 = fp8(VS*wv), wvr = fp8(VS*(wv - wv8/VS)).  All
  three terms accumulate into one psum group; the VS factor cancels
  exactly because the softmax-denominator column of vaug is memset to VS
  instead of 1.  More accurate than bf16 (residual compensation), 25%
  fewer PE cycles, and replaces the 4MB bf16 x DMA with a 2MB fp8 one.
- energy / AV / output projection run in bf16.

Scheduling (found via TimelineSim-driven search): per k-tile iteration
the PE emits strips -> v(kj+1) -> att_pu(kj-2)+norm -> proj(kj-5) ->
att_fin(kj-3, transposes) -> one 256-col q-or-k projection chunk.  The
qk-chunk drains sit last so their long wait on PE's qk matmuls cannot
head-of-line-block the per-tile normalize chain in the DVE queue.
Engine split: exps + y fc0 drains on Act; rec/ao, v/qk/y fc1 drains and
attT copies on DVE; band masks (batched across the 4 heads with a
multi-dim affine_select pattern) on Pool; all DMA issue on SP.  pu and
the two bitcast bf16 transpose slots share double-buffered PSUM banks
(mm 3 + pe 3 + ut 2 = 8).

Gotcha: bass float8e4 is IEEE e4m3 (max finite 240, exponent-15 encodes
inf/nan), NOT e4m3fn -- clip to +-240 before casting on the host.
"""

import ml_dtypes
import numpy as np

import concourse.bass as bass
import concourse.mybir as mybir
from concourse import bacc
from concourse.tile import TileContext
from concourse.bass_utils import run_bass_kernel_spmd
from concourse.masks import make_identity

B, N, E, H, DH, WIN = 2, 2048, 1024, 16, 64, 128
HPC = 4              # heads per core
SL = HPC * DH        # feature slice per core (256)
NT = N // 128        # 16 query/key tiles
F32 = mybir.dt.float32
BF16 = mybir.dt.bfloat16
FP8 = mybir.dt.float8e4
KO = E // 128        # 8 contraction tiles
WS = 32.0            # q/k weight pre-scale (keeps fp8 out of subnormals)
VS = 2048.0          # v weight/psum common scale; cancelled by the 2048
                     # softmax-denominator column in vaug
SCALE_E = (1.0 / 32.0) / (WS * WS)   # exp scale: 1/sqrt(E) / (32q * 32k)
# bass float8e4 is IEEE e4m3 (max finite 240, exponent-15 encodes inf/nan)
# -- NOT e4m3fn.  Clip before casting so tails don't become inf.
NP_FP8 = ml_dtypes.float8_e4m3
NP_BF16 = ml_dtypes.bfloat16


def _fp8(a):
    return np.clip(a, -240.0, 240.0).astype(NP_FP8)

_CACHED_NC = None

# build-time experiment knobs
CFG = {
    "strip_bufs": 6,
    "io_bufs": 6,
    "small_bufs": 6,
    "ps_e_bufs": 2,
    "ps_mm_bufs": 2,
    "exp_pair": False,
    "warmups": 12,
    "y_pair": True,
    "y_mode": "act_fc0",
    "attT_g0_act": False,
    "v_lead": 1,
    "qk_spread": "single12",
    "dma_order": "v1",
    "tail_y_split": True,
    "proj_lag": 5,
    "fin_lag": 1,
    # engine split knobs
    "qk_eng": ("dve", "dve", "act", "act", "act"),  # per qk chunk emission
    "y_fc_eng": ("dve", "dve"),
}


def _build_nc():
    nc = bacc.Bacc("TRN2", target_bir_lowering=False)
    DR = mybir.MatmulPerfMode.DoubleRow

    x8_d = nc.dram_tensor("x8", [128, KO, N], FP8, kind="ExternalInput")
    xr_d = nc.dram_tensor("xr", [128, KO, N], FP8, kind="ExternalInput")
    wq_d = nc.dram_tensor("wq8", [128, KO, SL], FP8, kind="ExternalInput")
    wk_d = nc.dram_tensor("wk8", [128, KO, SL], FP8, kind="ExternalInput")
    wv8_d = nc.dram_tensor("wv8", [128, KO, SL], FP8, kind="ExternalInput")
    wvr_d = nc.dram_tensor("wvr", [128, KO, SL], FP8, kind="ExternalInput")
    wp_d = nc.dram_tensor("wpb", [SL, E], BF16, kind="ExternalInput")
    if CFG.get("y_fp8", False) or CFG.get("y_fp8x", False):
        wph_d = nc.dram_tensor("wph", [SL, E], FP8, kind="ExternalInput")
        wpr_d = nc.dram_tensor("wpr", [SL, E], FP8, kind="ExternalInput")
    aux_d = nc.dram_tensor("aux", [128, 4], F32, kind="ExternalInput")
    y_d = nc.dram_tensor("y", [N, E], BF16, kind="ExternalOutput")

    with TileContext(nc) as tc:
        with (
            tc.tile_pool(name="const", bufs=1) as const,
            tc.tile_pool(name="persist", bufs=1) as persist,
            tc.tile_pool(name="io", bufs=CFG["io_bufs"]) as io,
            tc.tile_pool(name="small", bufs=CFG["small_bufs"]) as small,
            tc.tile_pool(name="strips", bufs=CFG["strip_bufs"]) as strip_pool,
            tc.tile_pool(name="ps_mm", bufs=(3 if not CFG.get("exp_pair", True) else CFG["ps_mm_bufs"]), space="PSUM") as ps_mm,
            tc.tile_pool(name="ps_e", bufs=(3 if not CFG.get("exp_pair", True) else CFG["ps_e_bufs"]), space="PSUM") as ps_e,
            tc.tile_pool(name="ps_ut", bufs=2, space="PSUM") as ps_ut,
        ):
            # ---- persistent SBUF tensors ----
            x8_sb = persist.tile([128, KO, N], FP8)
            xr_sb = persist.tile([128, KO, N], FP8)
            wq_sb = persist.tile([128, KO, SL], FP8)
            wk_sb = persist.tile([128, KO, SL], FP8)
            wv8_sb = persist.tile([128, KO, SL], FP8)
            wvr_sb = persist.tile([128, KO, SL], FP8)
            if CFG.get("y_fp8", False) or CFG.get("y_fp8x", False):
                wph_sb = persist.tile([128, 2, E], FP8)
                wpr_sb = persist.tile([128, 2, E], FP8)
            else:
                wp_sb = persist.tile([128, 2, E], BF16)
            aux = const.tile([128, 4], F32)

            def xc(sb, d, c0, c1, eng=None):  # column chunk of x8/xr
                (eng or nc.sync).dma_start(sb[:, :, c0:c1], d.ap()[:, :, c0:c1])

            # DMA order: feed q/k proj first, then v inputs, wp before
            # stage_proj(0) fires, rest of x by strip consumption order.
            if CFG.get("dma_order", "v2") in ("v4", "v5"):
                # first x8 chunk(s) via Pool SWDGE: lower launch latency and
                # a second issue queue for the prologue-critical bytes
                xc(x8_sb, x8_d, 0, 512, eng=nc.gpsimd)
                nc.sync.dma_start(wq_sb[:], wq_d.ap())
                nc.sync.dma_start(aux[:], aux_d.ap())
                nc.sync.dma_start(wk_sb[:], wk_d.ap())
                if CFG["dma_order"] == "v4":
                    xc(x8_sb, x8_d, 512, 1024, eng=nc.gpsimd)
                else:
                    xc(x8_sb, x8_d, 512, 1024)
            elif CFG.get("dma_order", "v2") == "v3":
                nc.sync.dma_start(wq_sb[:], wq_d.ap())
                xc(x8_sb, x8_d, 0, 128)
                nc.sync.dma_start(aux[:], aux_d.ap())
                nc.sync.dma_start(wk_sb[:], wk_d.ap())
                xc(x8_sb, x8_d, 128, 512)
                xc(x8_sb, x8_d, 512, 1024)
            elif CFG.get("dma_order", "v2") == "v2":
                xc(x8_sb, x8_d, 0, 128)
                nc.sync.dma_start(wq_sb[:], wq_d.ap())
                nc.sync.dma_start(aux[:], aux_d.ap())
                nc.sync.dma_start(wk_sb[:], wk_d.ap())
                xc(x8_sb, x8_d, 128, 640)
                xc(x8_sb, x8_d, 640, 1024)
            else:
                nc.sync.dma_start(wq_sb[:], wq_d.ap())
                xc(x8_sb, x8_d, 0, 512)
                nc.sync.dma_start(aux[:], aux_d.ap())
                nc.sync.dma_start(wk_sb[:], wk_d.ap())
                xc(x8_sb, x8_d, 512, 1024)
            nc.sync.dma_start(wv8_sb[:], wv8_d.ap())
            nc.sync.dma_start(wvr_sb[:], wvr_d.ap())
            xc(xr_sb, xr_d, 0, 512)
            xc(x8_sb, x8_d, 1024, 1536)
            xc(xr_sb, xr_d, 512, 1024)
            if CFG.get("y_fp8", False) or CFG.get("y_fp8x", False):
                nc.sync.dma_start(
                    wph_sb[:], wph_d.ap().rearrange("(g p) f -> p g f", p=128))
                nc.sync.dma_start(
                    wpr_sb[:], wpr_d.ap().rearrange("(g p) f -> p g f", p=128))
            else:
                nc.sync.dma_start(
                    wp_sb[:], wp_d.ap().rearrange("(g p) f -> p g f", p=128))
            xc(x8_sb, x8_d, 1536, 2048)
            xc(xr_sb, xr_d, 1024, 1536)
            xc(xr_sb, xr_d, 1536, 2048)

            bq_col = aux[:, 0:2]
            bk_col = aux[:, 2:4]

            # ---- on-chip constants ----
            warm = const.tile([128, 128], BF16)
            nc.gpsimd.memset(warm[:], 0.0)
            ident = const.tile([128, 128], BF16)
            make_identity(nc, ident[:])

            # ---- projection outputs ----
            qsb = persist.tile([128, 2, N], FP8, name="qsb")
            ksb = persist.tile([128, 2, N], FP8, name="ksb")
            vaug = persist.tile([128, NT, HPC, DH + 1], BF16)
            nc.gpsimd.memset(vaug[:, :, :, DH], float(VS))
            attT = persist.tile([128, 2, N], BF16, name="attT")
            if CFG.get("y_fp8", False) or CFG.get("y_fp8x", False):
                a16 = persist.tile([128, 2, N], FP8, name="a16")
                ar8 = persist.tile([128, 2, N], FP8, name="ar8")

            qk_emit_idx = [0]

            def emit_qk_chunk(cs, tensors=(0, 1)):
                w_cs = cs.stop - cs.start
                for ti, (w_sb, out_t, b_col) in enumerate(
                        ((wq_sb, qsb, bq_col), (wk_sb, ksb, bk_col))):
                    if ti not in tensors:
                        continue
                    if CFG.get("qk_merge_g", True) and w_cs <= 256:
                        # both g halves in one psum bank -> one drain
                        ps = ps_mm.tile([128, 2, 256], F32, tag="mm",
                                        name="ps_qk")
                        for g in range(2):
                            for kp in range(KO // 2):
                                nc.tensor.matmul(
                                    ps[:, g, :w_cs],
                                    lhsT=w_sb[:, 2 * kp:2 * kp + 2,
                                              g * 128:(g + 1) * 128],
                                    rhs=x8_sb[:, 2 * kp:2 * kp + 2, cs],
                                    start=(kp == 0),
                                    stop=(kp == KO // 2 - 1),
                                    perf_mode=DR)
                        nc.vector.tensor_tensor(
                            out_t[:, :, cs], ps[:, :, :w_cs],
                            b_col[:, :, None].broadcast_to([128, 2, w_cs]),
                            mybir.AluOpType.add)
                        continue
                    for g in range(2):
                        ps = ps_mm.tile([128, 512], F32, tag="mm", name="ps_qk")
                        ps = ps[:, :w_cs]
                        for kp in range(KO // 2):
                            nc.tensor.matmul(
                                ps,
                                lhsT=w_sb[:, 2 * kp:2 * kp + 2, g * 128:(g + 1) * 128],
                                rhs=x8_sb[:, 2 * kp:2 * kp + 2, cs],
                                start=(kp == 0), stop=(kp == KO // 2 - 1),
                                perf_mode=DR)
                        if CFG.get("qk_mix", False) and ti == 0:
                            nc.scalar.activation(
                                out_t[:, g, cs], ps,
                                mybir.ActivationFunctionType.Identity,
                                bias=b_col[:, g:g + 1])
                        else:
                            nc.vector.tensor_scalar_add(
                                out_t[:, g, cs], ps, b_col[:, g:g + 1])

            def emit_v_tile(nt):
                ps = ps_mm.tile([128, 512], F32, tag="mm", name="ps_v")
                psv = ps[:, :SL]
                rs = slice(nt * 128, (nt + 1) * 128)
                terms = ((x8_sb, wv8_sb), (xr_sb, wv8_sb), (x8_sb, wvr_sb))
                for ti, (xs_, ws_) in enumerate(terms):
                    for k2 in range(KO // 2):
                        nc.tensor.matmul(
                            psv, lhsT=xs_[:, 2 * k2:2 * k2 + 2, rs],
                            rhs=ws_[:, 2 * k2:2 * k2 + 2, :],
                            start=(ti == 0 and k2 == 0),
                            stop=(ti == 2 and k2 == KO // 2 - 1),
                            perf_mode=DR)
                if CFG.get("v_alt", False) and nt % 2 == 0:
                    nc.scalar.activation(
                        vaug[:, nt, :, :DH],
                        psv.rearrange("p (h d) -> p h d", d=DH),
                        mybir.ActivationFunctionType.Copy)
                else:
                    nc.vector.tensor_copy(
                        vaug[:, nt, :, :DH],
                        psv.rearrange("p (h d) -> p h d", d=DH))

            # ---- banded attention ----
            strips = {}

            def emit_strips(kj):
                lo, hi = max(0, kj - 1), min(NT - 1, kj + 1)
                w = (hi - lo + 1) * 128
                st4 = strip_pool.tile([128, HPC, 384], BF16, tag="strip",
                                      name="st4")
                if CFG.get("exp_pair", True):
                    for hp in range(HPC // 2):
                        # two heads share a 2-bank psum tile so the exp
                        # drains as one wide Activation instruction
                        pe = ps_e.tile([128, 2, 512], F32, tag="pe", name="pe")
                        for hh in range(2):
                            h = 2 * hp + hh
                            hb = 32 * h
                            nc.tensor.matmul(
                                pe[:, hh, :w],
                                lhsT=ksb[hb:hb + 32, :, kj * 128:(kj + 1) * 128],
                                rhs=qsb[hb:hb + 32, :, lo * 128:(hi + 1) * 128],
                                start=True, stop=True, perf_mode=DR,
                                tile_position=(hb, 0))
                        nc.scalar.activation(
                            st4[:, 2 * hp:2 * hp + 2, :w], pe[:, :, :w],
                            mybir.ActivationFunctionType.Exp, scale=SCALE_E)
                else:
                    for h in range(HPC):
                        pe = ps_e.tile([128, 384], F32, tag="pe", name="pe")
                        hb = 32 * h
                        nc.tensor.matmul(
                            pe[:, :w],
                            lhsT=ksb[hb:hb + 32, :, kj * 128:(kj + 1) * 128],
                            rhs=qsb[hb:hb + 32, :, lo * 128:(hi + 1) * 128],
                            start=True, stop=True, perf_mode=DR,
                            tile_position=(hb, 0))
                        nc.scalar.activation(
                            st4[:, h, :w], pe[:, :w],
                            mybir.ActivationFunctionType.Exp, scale=SCALE_E)
                # band masks, batched across the 4 heads (Pool, SBUF-only).
                # U block (q-tile kj-1): keep c >= p; L block: keep c <= p.
                if lo == kj - 1:
                    nc.gpsimd.affine_select(
                        out=st4[:, :, 0:128], in_=st4[:, :, 0:128],
                        compare_op=mybir.AluOpType.is_ge, fill=0.0, base=0,
                        pattern=[[0, HPC], [1, 128]], channel_multiplier=-1)
                if hi == kj + 1:
                    lc = (hi - lo) * 128
                    nc.gpsimd.affine_select(
                        out=st4[:, :, lc:lc + 128], in_=st4[:, :, lc:lc + 128],
                        compare_op=mybir.AluOpType.is_ge, fill=0.0, base=0,
                        pattern=[[0, HPC], [-1, 128]], channel_multiplier=1)
                strips[kj] = (st4, lo)

            att_state = {}

            def stage_att(t):
                stage_att_pu(t)
                stage_att_norm(t)
                stage_att_fin(t)

            def stage_att_pu(t):
                ks = [k for k in (t - 1, t, t + 1) if 0 <= k < NT]
                # pu (4*65 f32) plus two bitcast bf16 transpose slots share
                # each 1-bank ut tile; bufs=2 so tile t+1 never waits on
                # tile t's attT copies
                ut = ps_ut.tile([128, 512], F32, tag="ut", name="ut")
                pu = ut[:, 0:HPC * (DH + 1)].rearrange(
                    "p (h d) -> p h d", d=DH + 1)
                for h in range(HPC):
                    for i, k2 in enumerate(ks):
                        st4, lo2 = strips[k2]
                        col = (t - lo2) * 128
                        nc.tensor.matmul(
                            pu[:, h, :], lhsT=st4[:, h, col:col + 128],
                            rhs=vaug[:, k2, h, :],
                            start=(i == 0),
                            stop=(i == len(ks) - 1),
                            skip_group_check=True)
                att_state[t] = [ut, pu, None]

            def stage_att_norm(t):
                ut, pu, _ = att_state[t]
                ao = small.tile([128, HPC, DH], BF16, tag="ao", name="ao")
                if CFG.get("ao_div", False):
                    nc.vector.tensor_tensor(
                        ao[:], pu[:, :, :DH],
                        pu[:, :, DH:DH + 1].broadcast_to([128, HPC, DH]),
                        mybir.AluOpType.divide)
                else:
                    rec = small.tile([128, HPC], F32, tag="rec", name="rec")
                    nc.vector.reciprocal(rec[:], pu[:, :, DH])
                    nc.vector.tensor_mul(
                        ao[:], pu[:, :, :DH],
                        rec[:, :, None].broadcast_to([128, HPC, DH]))
                att_state[t][2] = ao

            def stage_att_fin(t):
                ts_ = slice(t * 128, (t + 1) * 128)
                ut, pu, ao = att_state.pop(t)
                if CFG.get("y_fp8x", False):
                    # SBUF->SBUF XBAR transpose; Pool then derives the fp8
                    # main+residual pair from attT without touching PSUM
                    nc.sync.dma_start_transpose(attT[:, :, ts_], ao[:])
                    nc.gpsimd.tensor_scalar_mul(
                        a16[:, :, ts_], attT[:, :, ts_], 16.0)
                    nc.gpsimd.scalar_tensor_tensor(
                        out=ar8[:, :, ts_], in0=attT[:, :, ts_],
                        scalar=16.0, in1=a16[:, :, ts_],
                        op0=mybir.AluOpType.mult,
                        op1=mybir.AluOpType.subtract)
                    return
                for g in range(2):
                    pt = ut[:, 320 + 64 * g:384 + 64 * g].bitcast(BF16)
                    nc.tensor.transpose(
                        pt, ao[:, 2 * g:2 * g + 2, :], ident[:])
                if CFG.get("attT_merge", True):
                    ptb = ut[:, 320:448].bitcast(BF16)
                    nc.vector.tensor_copy(
                        attT[:, :, ts_],
                        ptb.rearrange("p (g q) -> p g q", g=2))
                    if CFG.get("y_fp8", False):
                        nc.gpsimd.tensor_scalar_mul(
                            a16[:, :, ts_], attT[:, :, ts_], 16.0)
                        nc.gpsimd.scalar_tensor_tensor(
                            out=ar8[:, :, ts_], in0=attT[:, :, ts_],
                            scalar=16.0, in1=a16[:, :, ts_],
                            op0=mybir.AluOpType.mult,
                            op1=mybir.AluOpType.subtract)
                else:
                    for g in range(2):
                        pt = ut[:, 320 + 64 * g:384 + 64 * g].bitcast(BF16)
                        if g == 0 and CFG.get("attT_g0_act", False):
                            nc.scalar.activation(
                                attT[:, g, ts_], pt,
                                mybir.ActivationFunctionType.Copy)
                        else:
                            nc.vector.tensor_copy(attT[:, g, ts_], pt)

            ybuf = {}

            def stage_proj(t):
                ts_ = slice(t * 128, (t + 1) * 128)
                tail = t >= NT - 2 and CFG.get("tail_y_split", True)
                if CFG["y_pair"] and not tail:
                    if t % 2 == 0:
                        ybuf["t"] = io.tile([128, 2, E], BF16, tag="y2",
                                            name="y2_sb")
                    y_sb = ybuf["t"][:, t % 2, :]
                else:
                    y_sb = io.tile([128, E], BF16, tag="y", name="y_sb")
                for fc in range(2):
                    ps = ps_mm.tile([128, 512], F32, tag="mm", name="ps_yt")
                    fs = slice(fc * 512, (fc + 1) * 512)
                    if CFG.get("y_fp8", False) or CFG.get("y_fp8x", False):
                        terms = ((a16, wph_sb), (ar8, wph_sb), (a16, wpr_sb))
                        for ti2, (a_, w_) in enumerate(terms):
                            nc.tensor.matmul(
                                ps[:], lhsT=a_[:, :, ts_],
                                rhs=w_[:, :, fs],
                                start=(ti2 == 0), stop=(ti2 == 2),
                                perf_mode=DR)
                    else:
                        for g in range(2):
                            nc.tensor.matmul(
                                ps[:],
                                lhsT=attT[:, g, ts_],
                                rhs=wp_sb[:, g, fs],
                                start=(g == 0), stop=(g == 1))
                    ym = CFG.get("y_mode", "alt")
                    on_act = {"alt": (t + fc) % 2 == 0,
                              "dve": False,
                              "act_fc0": fc == 0,
                              "quarter": (t % 2 == 0) and fc == 0}[ym]
                    ysc = (1.0 / 2048.0
                           if CFG.get("y_fp8", False) or CFG.get("y_fp8x", False)
                           else None)
                    if on_act:
                        if ysc is None:
                            nc.scalar.activation(
                                y_sb[:, fs], ps[:],
                                mybir.ActivationFunctionType.Copy)
                        else:
                            nc.scalar.activation(
                                y_sb[:, fs], ps[:],
                                mybir.ActivationFunctionType.Identity,
                                scale=ysc)
                    elif ysc is None:
                        nc.vector.tensor_copy(y_sb[:, fs], ps[:])
                    else:
                        nc.vector.tensor_scalar_mul(y_sb[:, fs], ps[:], ysc)
                    if tail:
                        # drain latency off the critical tail: ship each
                        # 512-col half as soon as its copy lands
                        nc.sync.dma_start(y_d[ts_, fs], y_sb[:, fs])
                if tail:
                    pass
                elif CFG["y_pair"]:
                    if t % 2 == 1:
                        dst = y_d[(t - 1) * 128:(t + 1) * 128, :]
                        nc.sync.dma_start(
                            dst.rearrange("(tt p) f -> p tt f", p=128),
                            ybuf["t"][:])
                else:
                    nc.sync.dma_start(y_d[ts_, :], y_sb[:])

            # ---- schedule ----
            # PE warmup against the p-state ramp while input DMAs stream
            for i in range(CFG["warmups"]):
                if CFG.get("exp_pair", True):
                    pw = ps_e.tile([128, 2, 512], F32, tag="pe", name="pe_w")
                    pw = pw[:, 0, :128]
                else:
                    pw = ps_e.tile([128, 384], F32, tag="pe", name="pe_w")
                    pw = pw[:, :128]
                nc.tensor.matmul(pw, lhsT=warm[:], rhs=warm[:],
                                 start=True, stop=True)
            if CFG.get("qk_pre_merge", True):
                emit_qk_chunk(slice(0, 256))
                emit_qk_chunk(slice(256, 512))
            else:
                emit_qk_chunk(slice(0, 128))
                emit_qk_chunk(slice(128, 512))
            VL = CFG["v_lead"]
            for kj in range(NT + 1):
                if kj < NT:
                    emit_strips(kj)
                if not CFG.get("v_late", False):
                    if kj == 0:
                        for j in range(VL):
                            emit_v_tile(j)
                    if kj + VL < NT:
                        emit_v_tile(kj + VL)
                # steady lag 2/5; once strips end, drain without idle lag
                if CFG.get("v_late", False):
                    if kj == 0:
                        for j in range(VL):
                            emit_v_tile(j)
                    if kj + VL < NT:
                        emit_v_tile(kj + VL)
                FL = CFG.get("fin_lag", 1)
                if kj < NT:
                    if 2 <= kj < NT - 1:
                        stage_att_pu(kj - 2)
                        stage_att_norm(kj - 2)
                    elif kj == NT - 1:
                        for tt in (kj - 2, kj - 1, kj):
                            stage_att_pu(tt)
                            stage_att_norm(tt)
                        for tt in range(NT - 3 - FL, NT):
                            stage_att_fin(tt)
                PL = CFG["proj_lag"]
                sched = {
                    "A": {NT: list(range(NT - PL, NT))},
                    "B": {NT - 1: [NT - PL],
                          NT: list(range(NT - PL + 1, NT))},
                    "C": {NT - 2: [NT - PL], NT - 1: [NT - PL + 1],
                          NT: list(range(NT - PL + 2, NT))},
                }[CFG.get("tail_sched", "A")]
                if PL <= kj < NT and kj - PL in sched.get(kj, []):
                    raise AssertionError("dup proj")
                if PL <= kj < NT:
                    stage_proj(kj - PL)
                if 2 + FL <= kj < NT - 1:
                    stage_att_fin(kj - 2 - FL)
                for tt in sched.get(kj, []):
                    stage_proj(tt)
                # qk chunks go last: their drains must sit BEHIND rec/ao in
                # the DVE queue, else the long wait on PE's qk matmuls
                # head-of-line blocks the per-tile normalize chain
                if CFG.get("qk_spread", "single12") == "single12":
                    if 1 <= kj <= 12:
                        i = (kj - 1) // 2
                        emit_qk_chunk(slice(512 + i * 256, 768 + i * 256),
                                      tensors=((kj - 1) % 2,))
                else:
                    if kj in (1, 2, 4, 5, 8, 9):
                        i = (1, 2, 4, 5, 8, 9).index(kj)
                        emit_qk_chunk(slice(512 + i * 256, 768 + i * 256))

    nc.compile()
    return nc


def _get_nc():
    global _CACHED_NC
    if _CACHED_NC is None:
        _CACHED_NC = _build_nc()
    return _CACHED_NC


def _to_pm(a):
    """[E, X] -> partition-major [128, KO, X] (contiguous)."""
    return np.ascontiguousarray(
        a.reshape(KO, 128, a.shape[1]).transpose(1, 0, 2))


def kernel(x, Wq, bq, Wk, bk, Wv, bv, Wp, bp):
    nc = _get_nc()
    x = np.asarray(x, np.float32)
    Wq = np.asarray(Wq, np.float32)
    Wk = np.asarray(Wk, np.float32)
    Wv = np.asarray(Wv, np.float32)
    Wp = np.asarray(Wp, np.float32)
    bq = np.asarray(bq, np.float32)
    bk = np.asarray(bk, np.float32)
    bv = np.asarray(bv, np.float32)
    bp = np.asarray(bp, np.float32)

    # d-split DR layout: column j = i*128 + 32h + p  <->  feature
    # h*64 + i*32 + p
    j = np.arange(SL)
    f = (j % 128) // 32 * 64 + (j // 128) * 32 + (j % 32)

    xs = []
    for b in range(B):
        xT = np.ascontiguousarray(x[b].T)
        x8 = _fp8(xT)
        xr = _fp8(xT - x8.astype(np.float32))
        xs.append((_to_pm(x8), _to_pm(xr)))

    in_maps = []
    for c in range(8):
        b, gq = c // 4, c % 4
        sl = slice(SL * gq, SL * (gq + 1))
        wq_s = (WS * Wq[sl][f]).astype(np.float32)
        wk_s = (WS * Wk[sl][f]).astype(np.float32)
        bq_s = (WS * bq[sl][f]).astype(np.float32)
        bk_s = (WS * bk[sl][f]).astype(np.float32)
        aux = np.zeros((128, 4), np.float32)
        aux[:, 0] = bq_s[:128]
        aux[:, 1] = bq_s[128:]
        aux[:, 2] = bk_s[:128]
        aux[:, 3] = bk_s[128:]
        wvT = np.ascontiguousarray(Wv[sl].T)           # [E, SL]
        wv8 = _fp8(VS * wvT)                # stores 4096*wv
        wvr = _fp8((wvT - wv8.astype(np.float32) / VS) * VS)
        in_maps.append({
            "x8": xs[b][0],
            "xr": xs[b][1],
            "wq8": _to_pm(_fp8(np.ascontiguousarray(wq_s.T))),
            "wk8": _to_pm(_fp8(np.ascontiguousarray(wk_s.T))),
            "wv8": _to_pm(wv8),
            "wvr": _to_pm(wvr),
            "wpb": np.ascontiguousarray(Wp[:, sl].T).astype(NP_BF16),
            "aux": aux,
        })
    res = run_bass_kernel_spmd(nc, in_maps, core_ids=list(range(8)))
    ys = [np.asarray(res.results[c]["y"], np.float32) for c in range(8)]
    if any(not np.isfinite(y).all() for y in ys):
        # transient device flake observed once in ~15 runs; retry once
        res = run_bass_kernel_spmd(nc, in_maps, core_ids=list(range(8)))
        ys = [np.asarray(res.results[c]["y"], np.float32) for c in range(8)]
    # output bias: bp plus the folded v-bias contribution bv @ Wp^T
    # (exact because softmax rows sum to 1)
    ybias = bp + bv @ Wp.T
    y = np.stack([
        ys[0] + ys[1] + ys[2] + ys[3],
        ys[4] + ys[5] + ys[6] + ys[7],
    ]).astype(np.float32) + ybias[None, None, :]
    return y.astype(np.float32)


# revision 62
# speedup vs baseline: 1.0165x; 1.0165x over previous
"""Local (banded) attention kernel for Trainium2, sharded over 8 NeuronCores.

Sharding: core c handles batch b=c//4 and heads 4*(c%4)..4*(c%4)+3.
Host pre-transposes x and weight slices; device does QKV projection,
banded attention (window 128 -> only tile-diagonal +/-1 blocks), and the
per-core slice of the output projection. Host sums the 4 partial outputs
per batch and adds the output bias (including the folded V bias: since
softmax rows sum to 1, att@(v+bv) = att@v + bv, so bv@Wp^T moves to the
host-side output bias).

Mixed precision:
- q/k projections: fp8 DoubleRow with weights pre-scaled by 32 (keeps the
  small weights out of fp8's subnormal range); the energy exp scale
  absorbs the 32*32 factor.
- v projection: fp8 DoubleRow with residual compensation at one common
  scale VS=2048: v*VS = x8@wv8 + xr@wv8 + x8@wvr where x8 = fp8(x),
  xr = fp8(x - x8), wv8 = fp8(VS*wv), wvr = fp8(VS*(wv - wv8/VS)).  All
  three terms accumulate into one psum group; the VS factor cancels
  exactly because the softmax-denominator column of vaug is memset to VS
  instead of 1.  More accurate than bf16 (residual compensation), 25%
  fewer PE cycles, and replaces the 4MB bf16 x DMA with a 2MB fp8 one.
- energy / AV / output projection run in bf16.

Scheduling (found via TimelineSim-driven search): per k-tile iteration
the PE emits strips -> v(kj+1) -> att_pu(kj-2)+norm -> proj(kj-5) ->
att_fin(kj-3, transposes) -> one 256-col q-or-k projection chunk.  The
qk-chunk drains sit last so their long wait on PE's qk matmuls cannot
head-of-line-block the per-tile normalize chain in the DVE queue.
Engine split: exps + y fc0 drains on Act; rec/ao, v/qk/y fc1 drains and
attT copies on DVE; band masks (batched across the 4 heads with a
multi-dim affine_select pattern) on Pool; all DMA issue on SP.  pu and
the two bitcast bf16 transpose slots share double-buffered PSUM banks
(mm 3 + pe 3 + ut 2 = 8).

Gotcha: bass float8e4 is IEEE e4m3 (max finite 240, exponent-15 encodes
inf/nan), NOT e4m3fn -- clip to +-240 before casting on the host.
"""

import ml_dtypes
import numpy as np

import concourse.bass as bass
import concourse.mybir as mybir
from concourse import bacc
from concourse.tile import TileContext
from concourse.bass_utils import run_bass_kernel_spmd
from concourse.masks import make_identity

B, N, E, H, DH, WIN = 2, 2048, 1024, 16, 64, 128
HPC = 4              # heads per core
SL = HPC * DH        # feature slice per core (256)
NT = N // 128        # 16 query/key tiles
F32 = mybir.dt.float32
BF16 = mybir.dt.bfloat16
FP8 = mybir.dt.float8e4
KO = E // 128        # 8 contraction tiles
WS = 32.0            # q/k weight pre-scale (keeps fp8 out of subnormals)
VS = 2048.0          # v weight/psum common scale; cancelled by the 2048
                     # softmax-denominator column in vaug
SCALE_E = (1.0 / 32.0) / (WS * WS)   # exp scale: 1/sqrt(E) / (32q * 32k)
# bass float8e4 is IEEE e4m3 (max finite 240, exponent-15 encodes inf/nan)
# -- NOT e4m3fn.  Clip before casting so tails don't become inf.
NP_FP8 = ml_dtypes.float8_e4m3
NP_BF16 = ml_dtypes.bfloat16


def _fp8(a):
    return np.clip(a, -240.0, 240.0).astype(NP_FP8)

_CACHED_NC = None

# build-time experiment knobs
CFG = {
    "strip_bufs": 6,
    "io_bufs": 6,
    "small_bufs": 6,
    "ps_e_bufs": 2,
    "ps_mm_bufs": 2,
    "exp_pair": False,
    "warmups": 12,
    "y_pair": True,
    "y_mode": "act_fc0",
    "attT_g0_act": False,
    "v_lead": 1,
    "qk_spread": "single12",
    "dma_order": "v1",
    "tail_y_split": True,
    "proj_lag": 5,
    "fin_lag": 1,
    "attT_act_from": 14,
    "mask_split_from": 11,
    # engine split knobs
    "qk_eng": ("dve", "dve", "act", "act", "act"),  # per qk chunk emission
    "y_fc_eng": ("dve", "dve"),
}


def _build_nc():
    nc = bacc.Bacc("TRN2", target_bir_lowering=False)
    DR = mybir.MatmulPerfMode.DoubleRow

    x8_d = nc.dram_tensor("x8", [128, KO, N], FP8, kind="ExternalInput")
    x8f_d = nc.dram_tensor("x8f", [128, KO, 256], FP8, kind="ExternalInput")
    xr_d = nc.dram_tensor("xr", [128, KO, N], FP8, kind="ExternalInput")
    wq_d = nc.dram_tensor("wq8", [128, KO, SL], FP8, kind="ExternalInput")
    wk_d = nc.dram_tensor("wk8", [128, KO, SL], FP8, kind="ExternalInput")
    wv8_d = nc.dram_tensor("wv8", [128, KO, SL], FP8, kind="ExternalInput")
    wvr_d = nc.dram_tensor("wvr", [128, KO, SL], FP8, kind="ExternalInput")
    wp_d = nc.dram_tensor("wpb", [SL, E], BF16, kind="ExternalInput")
    if CFG.get("y_fp8", False) or CFG.get("y_fp8x", False):
        wph_d = nc.dram_tensor("wph", [SL, E], FP8, kind="ExternalInput")
        wpr_d = nc.dram_tensor("wpr", [SL, E], FP8, kind="ExternalInput")
    aux_d = nc.dram_tensor("aux", [128, 4], F32, kind="ExternalInput")
    y_d = nc.dram_tensor("y", [N, E], BF16, kind="ExternalOutput")

    with TileContext(nc) as tc:
        with (
            tc.tile_pool(name="const", bufs=1) as const,
            tc.tile_pool(name="persist", bufs=1) as persist,
            tc.tile_pool(name="io", bufs=CFG["io_bufs"]) as io,
            tc.tile_pool(name="small", bufs=CFG["small_bufs"]) as small,
            tc.tile_pool(name="strips", bufs=CFG["strip_bufs"]) as strip_pool,
            tc.tile_pool(name="ps_mm", bufs=(3 if not CFG.get("exp_pair", True) else CFG["ps_mm_bufs"]), space="PSUM") as ps_mm,
            tc.tile_pool(name="ps_e", bufs=(3 if not CFG.get("exp_pair", True) else CFG["ps_e_bufs"]), space="PSUM") as ps_e,
            tc.tile_pool(name="ps_ut", bufs=2, space="PSUM") as ps_ut,
        ):
            # ---- persistent SBUF tensors ----
            x8_sb = persist.tile([128, KO, N], FP8)
            x8f_sb = persist.tile([128, KO, 256], FP8)
            xr_sb = persist.tile([128, KO, N], FP8)
            wq_sb = persist.tile([128, KO, SL], FP8)
            wk_sb = persist.tile([128, KO, SL], FP8)
            wv8_sb = persist.tile([128, KO, SL], FP8)
            wvr_sb = persist.tile([128, KO, SL], FP8)
            if CFG.get("y_fp8", False) or CFG.get("y_fp8x", False):
                wph_sb = persist.tile([128, 2, E], FP8)
                wpr_sb = persist.tile([128, 2, E], FP8)
            else:
                wp_sb = persist.tile([128, 2, E], BF16)
            aux = const.tile([128, 4], F32)

            def xc(sb, d, c0, c1, eng=None):  # column chunk of x8/xr
                (eng or nc.sync).dma_start(sb[:, :, c0:c1], d.ap()[:, :, c0:c1])

            # DMA order: feed q/k proj first, then v inputs, wp before
            # stage_proj(0) fires, rest of x by strip consumption order.
            if CFG.get("dma_order", "v2") in ("v4", "v5"):
                # first x8 chunk(s) via Pool SWDGE: lower launch latency and
                # a second issue queue for the prologue-critical bytes
                xc(x8_sb, x8_d, 0, 512, eng=nc.gpsimd)
                nc.sync.dma_start(wq_sb[:], wq_d.ap())
                nc.sync.dma_start(aux[:], aux_d.ap())
                nc.sync.dma_start(wk_sb[:], wk_d.ap())
                if CFG["dma_order"] == "v4":
                    xc(x8_sb, x8_d, 512, 1024, eng=nc.gpsimd)
                else:
                    xc(x8_sb, x8_d, 512, 1024)
            elif CFG.get("dma_order", "v2") == "v3":
                nc.sync.dma_start(wq_sb[:], wq_d.ap())
                xc(x8_sb, x8_d, 0, 128)
                nc.sync.dma_start(aux[:], aux_d.ap())
                nc.sync.dma_start(wk_sb[:], wk_d.ap())
                xc(x8_sb, x8_d, 128, 512)
                xc(x8_sb, x8_d, 512, 1024)
            elif CFG.get("dma_order", "v2") == "v2":
                xc(x8_sb, x8_d, 0, 128)
                nc.sync.dma_start(wq_sb[:], wq_d.ap())
                nc.sync.dma_start(aux[:], aux_d.ap())
                nc.sync.dma_start(wk_sb[:], wk_d.ap())
                xc(x8_sb, x8_d, 128, 640)
                xc(x8_sb, x8_d, 640, 1024)
            elif CFG.get("x8_fast", False):
                nc.sync.dma_start(wq_sb[:], wq_d.ap())
                nc.sync.dma_start(x8f_sb[:], x8f_d.ap())
                nc.sync.dma_start(aux[:], aux_d.ap())
                nc.sync.dma_start(wk_sb[:], wk_d.ap())
                xc(x8_sb, x8_d, 0, 512)
                xc(x8_sb, x8_d, 512, 1024)
            else:
                nc.sync.dma_start(wq_sb[:], wq_d.ap())
                xc(x8_sb, x8_d, 0, 512)
                nc.sync.dma_start(aux[:], aux_d.ap())
                nc.sync.dma_start(wk_sb[:], wk_d.ap())
                xc(x8_sb, x8_d, 512, 1024)
            nc.sync.dma_start(wv8_sb[:], wv8_d.ap())
            nc.sync.dma_start(wvr_sb[:], wvr_d.ap())
            xc(xr_sb, xr_d, 0, 512)
            xc(x8_sb, x8_d, 1024, 1536)
            xc(xr_sb, xr_d, 512, 1024)
            if CFG.get("y_fp8", False) or CFG.get("y_fp8x", False):
                nc.sync.dma_start(
                    wph_sb[:], wph_d.ap().rearrange("(g p) f -> p g f", p=128))
                nc.sync.dma_start(
                    wpr_sb[:], wpr_d.ap().rearrange("(g p) f -> p g f", p=128))
            else:
                nc.sync.dma_start(
                    wp_sb[:], wp_d.ap().rearrange("(g p) f -> p g f", p=128))
            xc(x8_sb, x8_d, 1536, 2048)
            xc(xr_sb, xr_d, 1024, 1536)
            xc(xr_sb, xr_d, 1536, 2048)

            bq_col = aux[:, 0:2]
            bk_col = aux[:, 2:4]

            # ---- on-chip constants ----
            warm = const.tile([128, 128], BF16)
            nc.gpsimd.memset(warm[:], 0.0)
            ident = const.tile([128, 128], BF16)
            make_identity(nc, ident[:])

            # ---- projection outputs ----
            qsb = persist.tile([128, 2, N], FP8, name="qsb")
            ksb = persist.tile([128, 2, N], FP8, name="ksb")
            vaug = persist.tile([128, NT, HPC, DH + 1], BF16)
            nc.gpsimd.memset(vaug[:, :, :, DH], float(VS))
            attT = persist.tile([128, 2, N], BF16, name="attT")
            if CFG.get("y_fp8", False) or CFG.get("y_fp8x", False):
                a16 = persist.tile([128, 2, N], FP8, name="a16")
                ar8 = persist.tile([128, 2, N], FP8, name="ar8")

            qk_emit_idx = [0]

            def emit_qk_chunk(cs, tensors=(0, 1), pre=False, xf=None):
                w_cs = cs.stop - cs.start
                for ti, (w_sb, out_t, b_col) in enumerate(
                        ((wq_sb, qsb, bq_col), (wk_sb, ksb, bk_col))):
                    if ti not in tensors:
                        continue
                    if CFG.get("qk_merge_g", True) and w_cs <= 256:
                        # both g halves in one psum bank -> one drain
                        ps = ps_mm.tile([128, 2, 256], F32, tag="mm",
                                        name="ps_qk")
                        xs_src = xf if xf is not None else x8_sb
                        for g in range(2):
                            for kp in range(KO // 2):
                                nc.tensor.matmul(
                                    ps[:, g, :w_cs],
                                    lhsT=w_sb[:, 2 * kp:2 * kp + 2,
                                              g * 128:(g + 1) * 128],
                                    rhs=xs_src[:, 2 * kp:2 * kp + 2, cs],
                                    start=(kp == 0),
                                    stop=(kp == KO // 2 - 1),
                                    perf_mode=DR)
                        if ti == 1 and CFG.get("qk_pre_mix", False) and pre:
                            for g in range(2):
                                nc.scalar.activation(
                                    out_t[:, g, cs], ps[:, g, :w_cs],
                                    mybir.ActivationFunctionType.Identity,
                                    bias=b_col[:, g:g + 1])
                        else:
                            nc.vector.tensor_tensor(
                                out_t[:, :, cs], ps[:, :, :w_cs],
                                b_col[:, :, None].broadcast_to([128, 2, w_cs]),
                                mybir.AluOpType.add)
                        continue
                    for g in range(2):
                        ps = ps_mm.tile([128, 512], F32, tag="mm", name="ps_qk")
                        ps = ps[:, :w_cs]
                        for kp in range(KO // 2):
                            nc.tensor.matmul(
                                ps,
                                lhsT=w_sb[:, 2 * kp:2 * kp + 2, g * 128:(g + 1) * 128],
                                rhs=x8_sb[:, 2 * kp:2 * kp + 2, cs],
                                start=(kp == 0), stop=(kp == KO // 2 - 1),
                                perf_mode=DR)
                        if CFG.get("qk_mix", False) and ti == 0:
                            nc.scalar.activation(
                                out_t[:, g, cs], ps,
                                mybir.ActivationFunctionType.Identity,
                                bias=b_col[:, g:g + 1])
                        else:
                            nc.vector.tensor_scalar_add(
                                out_t[:, g, cs], ps, b_col[:, g:g + 1])

            def emit_v_tile(nt):
                ps = ps_mm.tile([128, 512], F32, tag="mm", name="ps_v")
                psv = ps[:, :SL]
                rs = slice(nt * 128, (nt + 1) * 128)
                terms = ((x8_sb, wv8_sb), (xr_sb, wv8_sb), (x8_sb, wvr_sb))
                for ti, (xs_, ws_) in enumerate(terms):
                    for k2 in range(KO // 2):
                        nc.tensor.matmul(
                            psv, lhsT=xs_[:, 2 * k2:2 * k2 + 2, rs],
                            rhs=ws_[:, 2 * k2:2 * k2 + 2, :],
                            start=(ti == 0 and k2 == 0),
                            stop=(ti == 2 and k2 == KO // 2 - 1),
                            perf_mode=DR)
                if CFG.get("v_alt", False) and nt % 2 == 0:
                    nc.scalar.activation(
                        vaug[:, nt, :, :DH],
                        psv.rearrange("p (h d) -> p h d", d=DH),
                        mybir.ActivationFunctionType.Copy)
                else:
                    nc.vector.tensor_copy(
                        vaug[:, nt, :, :DH],
                        psv.rearrange("p (h d) -> p h d", d=DH))

            # ---- banded attention ----
            strips = {}

            def emit_strips(kj):
                lo, hi = max(0, kj - 1), min(NT - 1, kj + 1)
                w = (hi - lo + 1) * 128
                st4 = strip_pool.tile([128, HPC, 384], BF16, tag="strip",
                                      name="st4")
                if CFG.get("exp_pair", True):
                    for hp in range(HPC // 2):
                        # two heads share a 2-bank psum tile so the exp
                        # drains as one wide Activation instruction
                        pe = ps_e.tile([128, 2, 512], F32, tag="pe", name="pe")
                        for hh in range(2):
                            h = 2 * hp + hh
                            hb = 32 * h
                            nc.tensor.matmul(
                                pe[:, hh, :w],
                                lhsT=ksb[hb:hb + 32, :, kj * 128:(kj + 1) * 128],
                                rhs=qsb[hb:hb + 32, :, lo * 128:(hi + 1) * 128],
                                start=True, stop=True, perf_mode=DR,
                                tile_position=(hb, 0))
                        nc.scalar.activation(
                            st4[:, 2 * hp:2 * hp + 2, :w], pe[:, :, :w],
                            mybir.ActivationFunctionType.Exp, scale=SCALE_E)
                else:
                    split = kj >= CFG.get("mask_split_from", 99)
                    for h in range(HPC):
                        pe = ps_e.tile([128, 384], F32, tag="pe", name="pe")
                        hb = 32 * h
                        nc.tensor.matmul(
                            pe[:, :w],
                            lhsT=ksb[hb:hb + 32, :, kj * 128:(kj + 1) * 128],
                            rhs=qsb[hb:hb + 32, :, lo * 128:(hi + 1) * 128],
                            start=True, stop=True, perf_mode=DR,
                            tile_position=(hb, 0))
                        nc.scalar.activation(
                            st4[:, h, :w], pe[:, :w],
                            mybir.ActivationFunctionType.Exp, scale=SCALE_E)
                        if split and h % 2 == 1:
                            emit_masks(st4, kj, lo, hi, slice(h - 1, h + 1), 2)
                if kj < CFG.get("mask_split_from", 99):
                    emit_masks(st4, kj, lo, hi, slice(0, HPC), HPC)
                strips[kj] = (st4, lo)

            def emit_masks(st4, kj, lo, hi, hs, nh):
                # band masks, batched across heads (Pool, SBUF-only).
                # U block (q-tile kj-1): keep c >= p; L block: keep c <= p.
                if lo == kj - 1:
                    nc.gpsimd.affine_select(
                        out=st4[:, hs, 0:128], in_=st4[:, hs, 0:128],
                        compare_op=mybir.AluOpType.is_ge, fill=0.0, base=0,
                        pattern=[[0, nh], [1, 128]], channel_multiplier=-1)
                if hi == kj + 1:
                    lc = (hi - lo) * 128
                    nc.gpsimd.affine_select(
                        out=st4[:, hs, lc:lc + 128], in_=st4[:, hs, lc:lc + 128],
                        compare_op=mybir.AluOpType.is_ge, fill=0.0, base=0,
                        pattern=[[0, nh], [-1, 128]], channel_multiplier=1)

            att_state = {}

            def stage_att(t):
                stage_att_pu(t)
                stage_att_norm(t)
                stage_att_fin(t)

            def stage_att_pu(t):
                ks = [k for k in (t - 1, t, t + 1) if 0 <= k < NT]
                # pu (4*65 f32) plus two bitcast bf16 transpose slots share
                # each 1-bank ut tile; bufs=2 so tile t+1 never waits on
                # tile t's attT copies
                ut = ps_ut.tile([128, 512], F32, tag="ut", name="ut")
                pu = ut[:, 0:HPC * (DH + 1)].rearrange(
                    "p (h d) -> p h d", d=DH + 1)
                for h in range(HPC):
                    for i, k2 in enumerate(ks):
                        st4, lo2 = strips[k2]
                        col = (t - lo2) * 128
                        nc.tensor.matmul(
                            pu[:, h, :], lhsT=st4[:, h, col:col + 128],
                            rhs=vaug[:, k2, h, :],
                            start=(i == 0),
                            stop=(i == len(ks) - 1),
                            skip_group_check=True)
                att_state[t] = [ut, pu, None]

            def stage_att_norm(t):
                ut, pu, _ = att_state[t]
                ao = small.tile([128, HPC, DH], BF16, tag="ao", name="ao")
                if CFG.get("ao_div", False):
                    nc.vector.tensor_tensor(
                        ao[:], pu[:, :, :DH],
                        pu[:, :, DH:DH + 1].broadcast_to([128, HPC, DH]),
                        mybir.AluOpType.divide)
                else:
                    rec = small.tile([128, HPC], F32, tag="rec", name="rec")
                    nc.vector.reciprocal(rec[:], pu[:, :, DH])
                    nc.vector.tensor_mul(
                        ao[:], pu[:, :, :DH],
                        rec[:, :, None].broadcast_to([128, HPC, DH]))
                att_state[t][2] = ao

            def stage_att_fin(t):
                ts_ = slice(t * 128, (t + 1) * 128)
                ut, pu, ao = att_state.pop(t)
                if CFG.get("y_fp8x", False):
                    # SBUF->SBUF XBAR transpose; Pool then derives the fp8
                    # main+residual pair from attT without touching PSUM
                    nc.sync.dma_start_transpose(attT[:, :, ts_], ao[:])
                    nc.gpsimd.tensor_scalar_mul(
                        a16[:, :, ts_], attT[:, :, ts_], 16.0)
                    nc.gpsimd.scalar_tensor_tensor(
                        out=ar8[:, :, ts_], in0=attT[:, :, ts_],
                        scalar=16.0, in1=a16[:, :, ts_],
                        op0=mybir.AluOpType.mult,
                        op1=mybir.AluOpType.subtract)
                    return
                for g in range(2):
                    pt = ut[:, 320 + 64 * g:384 + 64 * g].bitcast(BF16)
                    nc.tensor.transpose(
                        pt, ao[:, 2 * g:2 * g + 2, :], ident[:])
                if CFG.get("attT_merge", True):
                    ptb = ut[:, 320:448].bitcast(BF16)
                    if (t >= CFG.get("attT_act_from", 99)
                            or t < CFG.get("attT_act_until", 0)):
                        nc.scalar.activation(
                            attT[:, :, ts_],
                            ptb.rearrange("p (g q) -> p g q", g=2),
                            mybir.ActivationFunctionType.Copy)
                    else:
                        nc.vector.tensor_copy(
                            attT[:, :, ts_],
                            ptb.rearrange("p (g q) -> p g q", g=2))
                    if CFG.get("y_fp8", False):
                        nc.gpsimd.tensor_scalar_mul(
                            a16[:, :, ts_], attT[:, :, ts_], 16.0)
                        nc.gpsimd.scalar_tensor_tensor(
                            out=ar8[:, :, ts_], in0=attT[:, :, ts_],
                            scalar=16.0, in1=a16[:, :, ts_],
                            op0=mybir.AluOpType.mult,
                            op1=mybir.AluOpType.subtract)
                else:
                    for g in range(2):
                        pt = ut[:, 320 + 64 * g:384 + 64 * g].bitcast(BF16)
                        if g == 0 and CFG.get("attT_g0_act", False):
                            nc.scalar.activation(
                                attT[:, g, ts_], pt,
                                mybir.ActivationFunctionType.Copy)
                        else:
                            nc.vector.tensor_copy(attT[:, g, ts_], pt)

            ybuf = {}

            def stage_proj(t):
                ts_ = slice(t * 128, (t + 1) * 128)
                tail = t >= NT - 2 and CFG.get("tail_y_split", True)
                if CFG["y_pair"] and not tail:
                    if t % 2 == 0:
                        ybuf["t"] = io.tile([128, 2, E], BF16, tag="y2",
                                            name="y2_sb")
                    y_sb = ybuf["t"][:, t % 2, :]
                else:
                    y_sb = io.tile([128, E], BF16, tag="y", name="y_sb")
                for fc in range(2):
                    ps = ps_mm.tile([128, 512], F32, tag="mm", name="ps_yt")
                    fs = slice(fc * 512, (fc + 1) * 512)
                    if CFG.get("y_fp8", False) or CFG.get("y_fp8x", False):
                        terms = ((a16, wph_sb), (ar8, wph_sb), (a16, wpr_sb))
                        for ti2, (a_, w_) in enumerate(terms):
                            nc.tensor.matmul(
                                ps[:], lhsT=a_[:, :, ts_],
                                rhs=w_[:, :, fs],
                                start=(ti2 == 0), stop=(ti2 == 2),
                                perf_mode=DR)
                    else:
                        for g in range(2):
                            nc.tensor.matmul(
                                ps[:],
                                lhsT=attT[:, g, ts_],
                                rhs=wp_sb[:, g, fs],
                                start=(g == 0), stop=(g == 1))
                    ym = CFG.get("y_mode", "alt")
                    on_act = {"alt": (t + fc) % 2 == 0,
                              "dve": False,
                              "act_fc0": fc == 0,
                              "quarter": (t % 2 == 0) and fc == 0}[ym]
                    if t >= CFG.get("y_both_act_from", 99):
                        on_act = True
                    ysc = (1.0 / 2048.0
                           if CFG.get("y_fp8", False) or CFG.get("y_fp8x", False)
                           else None)
                    if on_act:
                        if ysc is None:
                            nc.scalar.activation(
                                y_sb[:, fs], ps[:],
                                mybir.ActivationFunctionType.Copy)
                        else:
                            nc.scalar.activation(
                                y_sb[:, fs], ps[:],
                                mybir.ActivationFunctionType.Identity,
                                scale=ysc)
                    elif ysc is None:
                        nc.vector.tensor_copy(y_sb[:, fs], ps[:])
                    else:
                        nc.vector.tensor_scalar_mul(y_sb[:, fs], ps[:], ysc)
                    if tail:
                        # drain latency off the critical tail: ship each
                        # 512-col half as soon as its copy lands
                        nc.sync.dma_start(y_d[ts_, fs], y_sb[:, fs])
                if tail:
                    pass
                elif CFG["y_pair"]:
                    if t % 2 == 1:
                        dst = y_d[(t - 1) * 128:(t + 1) * 128, :]
                        nc.sync.dma_start(
                            dst.rearrange("(tt p) f -> p tt f", p=128),
                            ybuf["t"][:])
                else:
                    nc.sync.dma_start(y_d[ts_, :], y_sb[:])

            # ---- schedule ----
            # PE warmup against the p-state ramp while input DMAs stream
            for i in range(CFG["warmups"]):
                if CFG.get("exp_pair", True):
                    pw = ps_e.tile([128, 2, 512], F32, tag="pe", name="pe_w")
                    pw = pw[:, 0, :128]
                else:
                    pw = ps_e.tile([128, 384], F32, tag="pe", name="pe_w")
                    pw = pw[:, :128]
                nc.tensor.matmul(pw, lhsT=warm[:], rhs=warm[:],
                                 start=True, stop=True)
            if CFG.get("qk_pre_merge", True):
                emit_qk_chunk(slice(0, 256), pre=True,
                              xf=(x8f_sb if CFG.get("x8_fast", False) else None))
                emit_qk_chunk(slice(256, 512), pre=True)
            else:
                emit_qk_chunk(slice(0, 128))
                emit_qk_chunk(slice(128, 512))
            VL = CFG["v_lead"]
            for kj in range(NT + 1):
                if kj < NT:
                    emit_strips(kj)
                if not CFG.get("v_late", False):
                    if kj == 0:
                        for j in range(VL):
                            emit_v_tile(j)
                    if kj + VL < NT:
                        emit_v_tile(kj + VL)
                # steady lag 2/5; once strips end, drain without idle lag
                if CFG.get("v_late", False):
                    if kj == 0:
                        for j in range(VL):
                            emit_v_tile(j)
                    if kj + VL < NT:
                        emit_v_tile(kj + VL)
                FL = CFG.get("fin_lag", 1)
                if kj < NT:
                    if 2 <= kj < NT - 1:
                        stage_att_pu(kj - 2)
                        stage_att_norm(kj - 2)
                    elif kj == NT - 1:
                        for tt in (kj - 2, kj - 1, kj):
                            stage_att_pu(tt)
                            stage_att_norm(tt)
                        for tt in range(NT - 3 - FL, NT):
                            stage_att_fin(tt)
                PL = CFG["proj_lag"]
                sched = {
                    "A": {NT: list(range(NT - PL, NT))},
                    "B": {NT - 1: [NT - PL],
                          NT: list(range(NT - PL + 1, NT))},
                    "C": {NT - 2: [NT - PL], NT - 1: [NT - PL + 1],
                          NT: list(range(NT - PL + 2, NT))},
                }[CFG.get("tail_sched", "A")]
                if PL <= kj < NT and kj - PL in sched.get(kj, []):
                    raise AssertionError("dup proj")
                if PL <= kj < NT:
                    stage_proj(kj - PL)
                if 2 + FL <= kj < NT - 1:
                    stage_att_fin(kj - 2 - FL)
                for tt in sched.get(kj, []):
                    stage_proj(tt)
                # qk chunks go last: their drains must sit BEHIND rec/ao in
                # the DVE queue, else the long wait on PE's qk matmuls
                # head-of-line blocks the per-tile normalize chain
                if CFG.get("qk_spread", "single12") == "single12":
                    if 1 <= kj <= 12:
                        i = (kj - 1) // 2
                        emit_qk_chunk(slice(512 + i * 256, 768 + i * 256),
                                      tensors=((kj - 1) % 2,))
                else:
                    if kj in (1, 2, 4, 5, 8, 9):
                        i = (1, 2, 4, 5, 8, 9).index(kj)
                        emit_qk_chunk(slice(512 + i * 256, 768 + i * 256))

    nc.compile()
    return nc


def _get_nc():
    global _CACHED_NC
    if _CACHED_NC is None:
        _CACHED_NC = _build_nc()
    return _CACHED_NC


def _to_pm(a):
    """[E, X] -> partition-major [128, KO, X] (contiguous)."""
    return np.ascontiguousarray(
        a.reshape(KO, 128, a.shape[1]).transpose(1, 0, 2))


def kernel(x, Wq, bq, Wk, bk, Wv, bv, Wp, bp):
    nc = _get_nc()
    x = np.asarray(x, np.float32)
    Wq = np.asarray(Wq, np.float32)
    Wk = np.asarray(Wk, np.float32)
    Wv = np.asarray(Wv, np.float32)
    Wp = np.asarray(Wp, np.float32)
    bq = np.asarray(bq, np.float32)
    bk = np.asarray(bk, np.float32)
    bv = np.asarray(bv, np.float32)
    bp = np.asarray(bp, np.float32)

    # d-split DR layout: column j = i*128 + 32h + p  <->  feature
    # h*64 + i*32 + p
    j = np.arange(SL)
    f = (j % 128) // 32 * 64 + (j // 128) * 32 + (j % 32)

    xs = []
    for b in range(B):
        xT = np.ascontiguousarray(x[b].T)
        x8 = _fp8(xT)
        xr = _fp8(xT - x8.astype(np.float32))
        xs.append((_to_pm(x8), _to_pm(xr)))

    in_maps = []
    for c in range(8):
        b, gq = c // 4, c % 4
        sl = slice(SL * gq, SL * (gq + 1))
        wq_s = (WS * Wq[sl][f]).astype(np.float32)
        wk_s = (WS * Wk[sl][f]).astype(np.float32)
        bq_s = (WS * bq[sl][f]).astype(np.float32)
        bk_s = (WS * bk[sl][f]).astype(np.float32)
        aux = np.zeros((128, 4), np.float32)
        aux[:, 0] = bq_s[:128]
        aux[:, 1] = bq_s[128:]
        aux[:, 2] = bk_s[:128]
        aux[:, 3] = bk_s[128:]
        wvT = np.ascontiguousarray(Wv[sl].T)           # [E, SL]
        wv8 = _fp8(VS * wvT)                # stores 4096*wv
        wvr = _fp8((wvT - wv8.astype(np.float32) / VS) * VS)
        in_maps.append({
            "x8": xs[b][0],
            "x8f": np.ascontiguousarray(xs[b][0][:, :, 0:256]),
            "xr": xs[b][1],
            "wq8": _to_pm(_fp8(np.ascontiguousarray(wq_s.T))),
            "wk8": _to_pm(_fp8(np.ascontiguousarray(wk_s.T))),
            "wv8": _to_pm(wv8),
            "wvr": _to_pm(wvr),
            "wpb": np.ascontiguousarray(Wp[:, sl].T).astype(NP_BF16),
            "aux": aux,
        })
    res = run_bass_kernel_spmd(nc, in_maps, core_ids=list(range(8)))
    ys = [np.asarray(res.results[c]["y"], np.float32) for c in range(8)]
    if any(not np.isfinite(y).all() for y in ys):
        # transient device flake observed once in ~15 runs; retry once
        res = run_bass_kernel_spmd(nc, in_maps, core_ids=list(range(8)))
        ys = [np.asarray(res.results[c]["y"], np.float32) for c in range(8)]
    # output bias: bp plus the folded v-bias contribution bv @ Wp^T
    # (exact because softmax rows sum to 1)
    ybias = bp + bv @ Wp.T
    y = np.stack([
        ys[0] + ys[1] + ys[2] + ys[3],
        ys[4] + ys[5] + ys[6] + ys[7],
    ]).astype(np.float32) + ybias[None, None, :]
    return y.astype(np.float32)


# revision 65
# speedup vs baseline: 1.0295x; 1.0128x over previous
"""Local (banded) attention kernel for Trainium2, sharded over 8 NeuronCores.

Sharding: core c handles batch b=c//4 and heads 4*(c%4)..4*(c%4)+3.
Host pre-transposes x and weight slices; device does QKV projection,
banded attention (window 128 -> only tile-diagonal +/-1 blocks), and the
per-core slice of the output projection. Host sums the 4 partial outputs
per batch and adds the output bias (including the folded V bias: since
softmax rows sum to 1, att@(v+bv) = att@v + bv, so bv@Wp^T moves to the
host-side output bias).

Mixed precision:
- q/k projections: fp8 DoubleRow with weights pre-scaled by 32 (keeps the
  small weights out of fp8's subnormal range); the energy exp scale
  absorbs the 32*32 factor.
- v projection: fp8 DoubleRow with residual compensation at one common
  scale VS=2048: v*VS = x8@wv8 + xr@wv8 + x8@wvr where x8 = fp8(x),
  xr = fp8(x - x8), wv8 = fp8(VS*wv), wvr = fp8(VS*(wv - wv8/VS)).  All
  three terms accumulate into one psum group; the VS factor cancels
  exactly because the softmax-denominator column of vaug is memset to VS
  instead of 1.  More accurate than bf16 (residual compensation), 25%
  fewer PE cycles, and replaces the 4MB bf16 x DMA with a 2MB fp8 one.
- energy / AV / output projection run in bf16.

Scheduling (found via TimelineSim-driven search): per k-tile iteration
the PE emits strips -> v(kj+1) -> att_pu(kj-2)+norm -> proj(kj-5) ->
att_fin(kj-3, transposes) -> one 256-col q-or-k projection chunk.  The
qk-chunk drains sit last so their long wait on PE's qk matmuls cannot
head-of-line-block the per-tile normalize chain in the DVE queue.
Engine split: exps + y fc0 drains on Act; rec/ao, v/qk/y fc1 drains and
attT copies on DVE; band masks (batched across the 4 heads with a
multi-dim affine_select pattern) on Pool; all DMA issue on SP.  pu and
the two bitcast bf16 transpose slots share double-buffered PSUM banks
(mm 3 + pe 3 + ut 2 = 8).

Gotcha: bass float8e4 is IEEE e4m3 (max finite 240, exponent-15 encodes
inf/nan), NOT e4m3fn -- clip to +-240 before casting on the host.
"""

import ml_dtypes
import numpy as np

import concourse.bass as bass
import concourse.mybir as mybir
from concourse import bacc
from concourse.tile import TileContext
from concourse.bass_utils import run_bass_kernel_spmd
from concourse.masks import make_identity

B, N, E, H, DH, WIN = 2, 2048, 1024, 16, 64, 128
HPC = 4              # heads per core
SL = HPC * DH        # feature slice per core (256)
NT = N // 128        # 16 query/key tiles
F32 = mybir.dt.float32
BF16 = mybir.dt.bfloat16
FP8 = mybir.dt.float8e4
KO = E // 128        # 8 contraction tiles
WS = 32.0            # q/k weight pre-scale (keeps fp8 out of subnormals)
VS = 2048.0          # v weight/psum common scale; cancelled by the 2048
                     # softmax-denominator column in vaug
SCALE_E = (1.0 / 32.0) / (WS * WS)   # exp scale: 1/sqrt(E) / (32q * 32k)
# bass float8e4 is IEEE e4m3 (max finite 240, exponent-15 encodes inf/nan)
# -- NOT e4m3fn.  Clip before casting so tails don't become inf.
NP_FP8 = ml_dtypes.float8_e4m3
NP_BF16 = ml_dtypes.bfloat16


def _fp8(a):
    return np.clip(a, -240.0, 240.0).astype(NP_FP8)

_CACHED_NC = None

# build-time experiment knobs
CFG = {
    "strip_bufs": 6,
    "io_bufs": 6,
    "small_bufs": 6,
    "ps_e_bufs": 2,
    "ps_mm_bufs": 2,
    "exp_pair": False,
    "warmups": 12,
    "y_pair": True,
    "y_mode": "act_fc0",
    "attT_g0_act": False,
    "v_lead": 1,
    "qk_spread": "single12",
    "dma_order": "v1",
    "tail_y_split": True,
    "proj_lag": 5,
    "fin_lag": 1,
    "attT_act_from": 14,
    "mask_split_from": 11,
    "tail_y_mode": "single",
    "tail_y_split_from": 12,
    # engine split knobs
    "qk_eng": ("dve", "dve", "act", "act", "act"),  # per qk chunk emission
    "y_fc_eng": ("dve", "dve"),
}


def _build_nc():
    nc = bacc.Bacc("TRN2", target_bir_lowering=False)
    DR = mybir.MatmulPerfMode.DoubleRow

    x8_d = nc.dram_tensor("x8", [128, KO, N], FP8, kind="ExternalInput")
    x8f_d = nc.dram_tensor("x8f", [128, KO, 256], FP8, kind="ExternalInput")
    xr_d = nc.dram_tensor("xr", [128, KO, N], FP8, kind="ExternalInput")
    wq_d = nc.dram_tensor("wq8", [128, KO, SL], FP8, kind="ExternalInput")
    wk_d = nc.dram_tensor("wk8", [128, KO, SL], FP8, kind="ExternalInput")
    wv8_d = nc.dram_tensor("wv8", [128, KO, SL], FP8, kind="ExternalInput")
    wvr_d = nc.dram_tensor("wvr", [128, KO, SL], FP8, kind="ExternalInput")
    wp_d = nc.dram_tensor("wpb", [SL, E], BF16, kind="ExternalInput")
    if CFG.get("y_fp8", False) or CFG.get("y_fp8x", False):
        wph_d = nc.dram_tensor("wph", [SL, E], FP8, kind="ExternalInput")
        wpr_d = nc.dram_tensor("wpr", [SL, E], FP8, kind="ExternalInput")
    aux_d = nc.dram_tensor("aux", [128, 4], F32, kind="ExternalInput")
    y_d = nc.dram_tensor("y", [N, E], BF16, kind="ExternalOutput")

    with TileContext(nc) as tc:
        with (
            tc.tile_pool(name="const", bufs=1) as const,
            tc.tile_pool(name="persist", bufs=1) as persist,
            tc.tile_pool(name="io", bufs=CFG["io_bufs"]) as io,
            tc.tile_pool(name="small", bufs=CFG["small_bufs"]) as small,
            tc.tile_pool(name="strips", bufs=CFG["strip_bufs"]) as strip_pool,
            tc.tile_pool(name="ps_mm", bufs=(3 if not CFG.get("exp_pair", True) else CFG["ps_mm_bufs"]), space="PSUM") as ps_mm,
            tc.tile_pool(name="ps_e", bufs=(3 if not CFG.get("exp_pair", True) else CFG["ps_e_bufs"]), space="PSUM") as ps_e,
            tc.tile_pool(name="ps_ut", bufs=2, space="PSUM") as ps_ut,
        ):
            # ---- persistent SBUF tensors ----
            x8_sb = persist.tile([128, KO, N], FP8)
            x8f_sb = persist.tile([128, KO, 256], FP8)
            xr_sb = persist.tile([128, KO, N], FP8)
            wq_sb = persist.tile([128, KO, SL], FP8)
            wk_sb = persist.tile([128, KO, SL], FP8)
            wv8_sb = persist.tile([128, KO, SL], FP8)
            wvr_sb = persist.tile([128, KO, SL], FP8)
            if CFG.get("y_fp8", False) or CFG.get("y_fp8x", False):
                wph_sb = persist.tile([128, 2, E], FP8)
                wpr_sb = persist.tile([128, 2, E], FP8)
            else:
                wp_sb = persist.tile([128, 2, E], BF16)
            aux = const.tile([128, 4], F32)

            def xc(sb, d, c0, c1, eng=None):  # column chunk of x8/xr
                (eng or nc.sync).dma_start(sb[:, :, c0:c1], d.ap()[:, :, c0:c1])

            # DMA order: feed q/k proj first, then v inputs, wp before
            # stage_proj(0) fires, rest of x by strip consumption order.
            if CFG.get("dma_order", "v2") in ("v4", "v5"):
                # first x8 chunk(s) via Pool SWDGE: lower launch latency and
                # a second issue queue for the prologue-critical bytes
                xc(x8_sb, x8_d, 0, 512, eng=nc.gpsimd)
                nc.sync.dma_start(wq_sb[:], wq_d.ap())
                nc.sync.dma_start(aux[:], aux_d.ap())
                nc.sync.dma_start(wk_sb[:], wk_d.ap())
                if CFG["dma_order"] == "v4":
                    xc(x8_sb, x8_d, 512, 1024, eng=nc.gpsimd)
                else:
                    xc(x8_sb, x8_d, 512, 1024)
            elif CFG.get("dma_order", "v2") == "v3":
                nc.sync.dma_start(wq_sb[:], wq_d.ap())
                xc(x8_sb, x8_d, 0, 128)
                nc.sync.dma_start(aux[:], aux_d.ap())
                nc.sync.dma_start(wk_sb[:], wk_d.ap())
                xc(x8_sb, x8_d, 128, 512)
                xc(x8_sb, x8_d, 512, 1024)
            elif CFG.get("dma_order", "v2") == "v2":
                xc(x8_sb, x8_d, 0, 128)
                nc.sync.dma_start(wq_sb[:], wq_d.ap())
                nc.sync.dma_start(aux[:], aux_d.ap())
                nc.sync.dma_start(wk_sb[:], wk_d.ap())
                xc(x8_sb, x8_d, 128, 640)
                xc(x8_sb, x8_d, 640, 1024)
            elif CFG.get("x8_fast", False):
                nc.sync.dma_start(wq_sb[:], wq_d.ap())
                nc.sync.dma_start(x8f_sb[:], x8f_d.ap())
                nc.sync.dma_start(aux[:], aux_d.ap())
                nc.sync.dma_start(wk_sb[:], wk_d.ap())
                xc(x8_sb, x8_d, 0, 512)
                xc(x8_sb, x8_d, 512, 1024)
            else:
                nc.sync.dma_start(wq_sb[:], wq_d.ap())
                xc(x8_sb, x8_d, 0, 512)
                nc.sync.dma_start(aux[:], aux_d.ap())
                nc.sync.dma_start(wk_sb[:], wk_d.ap())
                xc(x8_sb, x8_d, 512, 1024)
            nc.sync.dma_start(wv8_sb[:], wv8_d.ap())
            nc.sync.dma_start(wvr_sb[:], wvr_d.ap())
            xc(xr_sb, xr_d, 0, 512)
            xc(x8_sb, x8_d, 1024, 1536)
            xc(xr_sb, xr_d, 512, 1024)
            if CFG.get("y_fp8", False) or CFG.get("y_fp8x", False):
                nc.sync.dma_start(
                    wph_sb[:], wph_d.ap().rearrange("(g p) f -> p g f", p=128))
                nc.sync.dma_start(
                    wpr_sb[:], wpr_d.ap().rearrange("(g p) f -> p g f", p=128))
            else:
                nc.sync.dma_start(
                    wp_sb[:], wp_d.ap().rearrange("(g p) f -> p g f", p=128))
            xc(x8_sb, x8_d, 1536, 2048)
            xc(xr_sb, xr_d, 1024, 1536)
            xc(xr_sb, xr_d, 1536, 2048)

            bq_col = aux[:, 0:2]
            bk_col = aux[:, 2:4]

            # ---- on-chip constants ----
            warm = const.tile([128, 128], BF16)
            nc.gpsimd.memset(warm[:], 0.0)
            ident = const.tile([128, 128], BF16)
            make_identity(nc, ident[:])

            # ---- projection outputs ----
            qsb = persist.tile([128, 2, N], FP8, name="qsb")
            ksb = persist.tile([128, 2, N], FP8, name="ksb")
            vaug = persist.tile([128, NT, HPC, DH + 1], BF16)
            nc.gpsimd.memset(vaug[:, :, :, DH], float(VS))
            attT = persist.tile([128, 2, N], BF16, name="attT")
            if CFG.get("y_fp8", False) or CFG.get("y_fp8x", False):
                a16 = persist.tile([128, 2, N], FP8, name="a16")
                ar8 = persist.tile([128, 2, N], FP8, name="ar8")

            qk_emit_idx = [0]

            def emit_qk_chunk(cs, tensors=(0, 1), pre=False, xf=None):
                w_cs = cs.stop - cs.start
                for ti, (w_sb, out_t, b_col) in enumerate(
                        ((wq_sb, qsb, bq_col), (wk_sb, ksb, bk_col))):
                    if ti not in tensors:
                        continue
                    if CFG.get("qk_merge_g", True) and w_cs <= 256:
                        # both g halves in one psum bank -> one drain
                        ps = ps_mm.tile([128, 2, 256], F32, tag="mm",
                                        name="ps_qk")
                        xs_src = xf if xf is not None else x8_sb
                        for g in range(2):
                            for kp in range(KO // 2):
                                nc.tensor.matmul(
                                    ps[:, g, :w_cs],
                                    lhsT=w_sb[:, 2 * kp:2 * kp + 2,
                                              g * 128:(g + 1) * 128],
                                    rhs=xs_src[:, 2 * kp:2 * kp + 2, cs],
                                    start=(kp == 0),
                                    stop=(kp == KO // 2 - 1),
                                    perf_mode=DR)
                        if ti == 1 and CFG.get("qk_pre_mix", False) and pre:
                            for g in range(2):
                                nc.scalar.activation(
                                    out_t[:, g, cs], ps[:, g, :w_cs],
                                    mybir.ActivationFunctionType.Identity,
                                    bias=b_col[:, g:g + 1])
                        else:
                            nc.vector.tensor_tensor(
                                out_t[:, :, cs], ps[:, :, :w_cs],
                                b_col[:, :, None].broadcast_to([128, 2, w_cs]),
                                mybir.AluOpType.add)
                        continue
                    for g in range(2):
                        ps = ps_mm.tile([128, 512], F32, tag="mm", name="ps_qk")
                        ps = ps[:, :w_cs]
                        for kp in range(KO // 2):
                            nc.tensor.matmul(
                                ps,
                                lhsT=w_sb[:, 2 * kp:2 * kp + 2, g * 128:(g + 1) * 128],
                                rhs=x8_sb[:, 2 * kp:2 * kp + 2, cs],
                                start=(kp == 0), stop=(kp == KO // 2 - 1),
                                perf_mode=DR)
                        if CFG.get("qk_mix", False) and ti == 0:
                            nc.scalar.activation(
                                out_t[:, g, cs], ps,
                                mybir.ActivationFunctionType.Identity,
                                bias=b_col[:, g:g + 1])
                        else:
                            nc.vector.tensor_scalar_add(
                                out_t[:, g, cs], ps, b_col[:, g:g + 1])

            def emit_v_tile(nt):
                ps = ps_mm.tile([128, 512], F32, tag="mm", name="ps_v")
                psv = ps[:, :SL]
                rs = slice(nt * 128, (nt + 1) * 128)
                terms = ((x8_sb, wv8_sb), (xr_sb, wv8_sb), (x8_sb, wvr_sb))
                for ti, (xs_, ws_) in enumerate(terms):
                    for k2 in range(KO // 2):
                        nc.tensor.matmul(
                            psv, lhsT=xs_[:, 2 * k2:2 * k2 + 2, rs],
                            rhs=ws_[:, 2 * k2:2 * k2 + 2, :],
                            start=(ti == 0 and k2 == 0),
                            stop=(ti == 2 and k2 == KO // 2 - 1),
                            perf_mode=DR)
                if CFG.get("v_alt", False) and nt % 2 == 0:
                    nc.scalar.activation(
                        vaug[:, nt, :, :DH],
                        psv.rearrange("p (h d) -> p h d", d=DH),
                        mybir.ActivationFunctionType.Copy)
                else:
                    nc.vector.tensor_copy(
                        vaug[:, nt, :, :DH],
                        psv.rearrange("p (h d) -> p h d", d=DH))

            # ---- banded attention ----
            strips = {}

            def emit_strips(kj):
                lo, hi = max(0, kj - 1), min(NT - 1, kj + 1)
                w = (hi - lo + 1) * 128
                st4 = strip_pool.tile([128, HPC, 384], BF16, tag="strip",
                                      name="st4")
                if CFG.get("exp_pair", True):
                    for hp in range(HPC // 2):
                        # two heads share a 2-bank psum tile so the exp
                        # drains as one wide Activation instruction
                        pe = ps_e.tile([128, 2, 512], F32, tag="pe", name="pe")
                        for hh in range(2):
                            h = 2 * hp + hh
                            hb = 32 * h
                            nc.tensor.matmul(
                                pe[:, hh, :w],
                                lhsT=ksb[hb:hb + 32, :, kj * 128:(kj + 1) * 128],
                                rhs=qsb[hb:hb + 32, :, lo * 128:(hi + 1) * 128],
                                start=True, stop=True, perf_mode=DR,
                                tile_position=(hb, 0))
                        nc.scalar.activation(
                            st4[:, 2 * hp:2 * hp + 2, :w], pe[:, :, :w],
                            mybir.ActivationFunctionType.Exp, scale=SCALE_E)
                else:
                    split = kj >= CFG.get("mask_split_from", 99)
                    for h in range(HPC):
                        pe = ps_e.tile([128, 384], F32, tag="pe", name="pe")
                        hb = 32 * h
                        nc.tensor.matmul(
                            pe[:, :w],
                            lhsT=ksb[hb:hb + 32, :, kj * 128:(kj + 1) * 128],
                            rhs=qsb[hb:hb + 32, :, lo * 128:(hi + 1) * 128],
                            start=True, stop=True, perf_mode=DR,
                            tile_position=(hb, 0))
                        nc.scalar.activation(
                            st4[:, h, :w], pe[:, :w],
                            mybir.ActivationFunctionType.Exp, scale=SCALE_E)
                        if split and h % 2 == 1:
                            emit_masks(st4, kj, lo, hi, slice(h - 1, h + 1), 2)
                if kj < CFG.get("mask_split_from", 99):
                    emit_masks(st4, kj, lo, hi, slice(0, HPC), HPC)
                strips[kj] = (st4, lo)

            def emit_masks(st4, kj, lo, hi, hs, nh):
                # band masks, batched across heads (Pool, SBUF-only).
                # U block (q-tile kj-1): keep c >= p; L block: keep c <= p.
                if lo == kj - 1:
                    nc.gpsimd.affine_select(
                        out=st4[:, hs, 0:128], in_=st4[:, hs, 0:128],
                        compare_op=mybir.AluOpType.is_ge, fill=0.0, base=0,
                        pattern=[[0, nh], [1, 128]], channel_multiplier=-1)
                if hi == kj + 1:
                    lc = (hi - lo) * 128
                    nc.gpsimd.affine_select(
                        out=st4[:, hs, lc:lc + 128], in_=st4[:, hs, lc:lc + 128],
                        compare_op=mybir.AluOpType.is_ge, fill=0.0, base=0,
                        pattern=[[0, nh], [-1, 128]], channel_multiplier=1)

            att_state = {}

            def stage_att(t):
                stage_att_pu(t)
                stage_att_norm(t)
                stage_att_fin(t)

            def stage_att_pu(t):
                ks = [k for k in (t - 1, t, t + 1) if 0 <= k < NT]
                # pu (4*65 f32) plus two bitcast bf16 transpose slots share
                # each 1-bank ut tile; bufs=2 so tile t+1 never waits on
                # tile t's attT copies
                ut = ps_ut.tile([128, 512], F32, tag="ut", name="ut")
                pu = ut[:, 0:HPC * (DH + 1)].rearrange(
                    "p (h d) -> p h d", d=DH + 1)
                for h in range(HPC):
                    for i, k2 in enumerate(ks):
                        st4, lo2 = strips[k2]
                        col = (t - lo2) * 128
                        nc.tensor.matmul(
                            pu[:, h, :], lhsT=st4[:, h, col:col + 128],
                            rhs=vaug[:, k2, h, :],
                            start=(i == 0),
                            stop=(i == len(ks) - 1),
                            skip_group_check=True)
                att_state[t] = [ut, pu, None]

            def stage_att_norm(t):
                ut, pu, _ = att_state[t]
                ao = small.tile([128, HPC, DH], BF16, tag="ao", name="ao")
                if CFG.get("ao_div", False):
                    nc.vector.tensor_tensor(
                        ao[:], pu[:, :, :DH],
                        pu[:, :, DH:DH + 1].broadcast_to([128, HPC, DH]),
                        mybir.AluOpType.divide)
                else:
                    rec = small.tile([128, HPC], F32, tag="rec", name="rec")
                    nc.vector.reciprocal(rec[:], pu[:, :, DH])
                    nc.vector.tensor_mul(
                        ao[:], pu[:, :, :DH],
                        rec[:, :, None].broadcast_to([128, HPC, DH]))
                att_state[t][2] = ao

            def stage_att_fin(t):
                ts_ = slice(t * 128, (t + 1) * 128)
                ut, pu, ao = att_state.pop(t)
                if CFG.get("y_fp8x", False):
                    # SBUF->SBUF XBAR transpose; Pool then derives the fp8
                    # main+residual pair from attT without touching PSUM
                    nc.sync.dma_start_transpose(attT[:, :, ts_], ao[:])
                    nc.gpsimd.tensor_scalar_mul(
                        a16[:, :, ts_], attT[:, :, ts_], 16.0)
                    nc.gpsimd.scalar_tensor_tensor(
                        out=ar8[:, :, ts_], in0=attT[:, :, ts_],
                        scalar=16.0, in1=a16[:, :, ts_],
                        op0=mybir.AluOpType.mult,
                        op1=mybir.AluOpType.subtract)
                    return
                for g in range(2):
                    pt = ut[:, 320 + 64 * g:384 + 64 * g].bitcast(BF16)
                    nc.tensor.transpose(
                        pt, ao[:, 2 * g:2 * g + 2, :], ident[:])
                if CFG.get("attT_merge", True):
                    ptb = ut[:, 320:448].bitcast(BF16)
                    if (t >= CFG.get("attT_act_from", 99)
                            or t < CFG.get("attT_act_until", 0)):
                        nc.scalar.activation(
                            attT[:, :, ts_],
                            ptb.rearrange("p (g q) -> p g q", g=2),
                            mybir.ActivationFunctionType.Copy)
                    else:
                        nc.vector.tensor_copy(
                            attT[:, :, ts_],
                            ptb.rearrange("p (g q) -> p g q", g=2))
                    if CFG.get("y_fp8", False):
                        nc.gpsimd.tensor_scalar_mul(
                            a16[:, :, ts_], attT[:, :, ts_], 16.0)
                        nc.gpsimd.scalar_tensor_tensor(
                            out=ar8[:, :, ts_], in0=attT[:, :, ts_],
                            scalar=16.0, in1=a16[:, :, ts_],
                            op0=mybir.AluOpType.mult,
                            op1=mybir.AluOpType.subtract)
                else:
                    for g in range(2):
                        pt = ut[:, 320 + 64 * g:384 + 64 * g].bitcast(BF16)
                        if g == 0 and CFG.get("attT_g0_act", False):
                            nc.scalar.activation(
                                attT[:, g, ts_], pt,
                                mybir.ActivationFunctionType.Copy)
                        else:
                            nc.vector.tensor_copy(attT[:, g, ts_], pt)

            ybuf = {}

            def stage_proj(t):
                ts_ = slice(t * 128, (t + 1) * 128)
                tail = (t >= CFG.get("tail_y_split_from", NT - 2)
                        and CFG.get("tail_y_split", True))
                if CFG["y_pair"] and not tail:
                    if t % 2 == 0:
                        ybuf["t"] = io.tile([128, 2, E], BF16, tag="y2",
                                            name="y2_sb")
                    y_sb = ybuf["t"][:, t % 2, :]
                else:
                    y_sb = io.tile([128, E], BF16, tag="y", name="y_sb")
                for fc in range(2):
                    ps = ps_mm.tile([128, 512], F32, tag="mm", name="ps_yt")
                    fs = slice(fc * 512, (fc + 1) * 512)
                    if CFG.get("y_fp8", False) or CFG.get("y_fp8x", False):
                        terms = ((a16, wph_sb), (ar8, wph_sb), (a16, wpr_sb))
                        for ti2, (a_, w_) in enumerate(terms):
                            nc.tensor.matmul(
                                ps[:], lhsT=a_[:, :, ts_],
                                rhs=w_[:, :, fs],
                                start=(ti2 == 0), stop=(ti2 == 2),
                                perf_mode=DR)
                    else:
                        for g in range(2):
                            nc.tensor.matmul(
                                ps[:],
                                lhsT=attT[:, g, ts_],
                                rhs=wp_sb[:, g, fs],
                                start=(g == 0), stop=(g == 1))
                    ym = CFG.get("y_mode", "alt")
                    on_act = {"alt": (t + fc) % 2 == 0,
                              "dve": False,
                              "act_fc0": fc == 0,
                              "quarter": (t % 2 == 0) and fc == 0}[ym]
                    if t >= CFG.get("y_both_act_from", 99):
                        on_act = True
                    ysc = (1.0 / 2048.0
                           if CFG.get("y_fp8", False) or CFG.get("y_fp8x", False)
                           else None)
                    if on_act:
                        if ysc is None:
                            nc.scalar.activation(
                                y_sb[:, fs], ps[:],
                                mybir.ActivationFunctionType.Copy)
                        else:
                            nc.scalar.activation(
                                y_sb[:, fs], ps[:],
                                mybir.ActivationFunctionType.Identity,
                                scale=ysc)
                    elif ysc is None:
                        nc.vector.tensor_copy(y_sb[:, fs], ps[:])
                    else:
                        nc.vector.tensor_scalar_mul(y_sb[:, fs], ps[:], ysc)
                    if tail and CFG.get("tail_y_mode", "fc") == "fc":
                        # drain latency off the critical tail: ship each
                        # 512-col half as soon as its copy lands
                        nc.sync.dma_start(y_d[ts_, fs], y_sb[:, fs])
                if tail:
                    if CFG.get("tail_y_mode", "fc") == "single":
                        nc.sync.dma_start(y_d[ts_, :], y_sb[:])
                elif CFG["y_pair"]:
                    if t % 2 == 1:
                        dst = y_d[(t - 1) * 128:(t + 1) * 128, :]
                        nc.sync.dma_start(
                            dst.rearrange("(tt p) f -> p tt f", p=128),
                            ybuf["t"][:])
                else:
                    nc.sync.dma_start(y_d[ts_, :], y_sb[:])

            # ---- schedule ----
            # PE warmup against the p-state ramp while input DMAs stream
            for i in range(CFG["warmups"]):
                if CFG.get("exp_pair", True):
                    pw = ps_e.tile([128, 2, 512], F32, tag="pe", name="pe_w")
                    pw = pw[:, 0, :128]
                else:
                    pw = ps_e.tile([128, 384], F32, tag="pe", name="pe_w")
                    pw = pw[:, :128]
                nc.tensor.matmul(pw, lhsT=warm[:], rhs=warm[:],
                                 start=True, stop=True)
            if CFG.get("qk_pre_merge", True):
                emit_qk_chunk(slice(0, 256), pre=True,
                              xf=(x8f_sb if CFG.get("x8_fast", False) else None))
                emit_qk_chunk(slice(256, 512), pre=True)
            else:
                emit_qk_chunk(slice(0, 128))
                emit_qk_chunk(slice(128, 512))
            VL = CFG["v_lead"]
            for kj in range(NT + 1):
                if kj < NT:
                    emit_strips(kj)
                if not CFG.get("v_late", False):
                    if kj == 0:
                        for j in range(VL):
                            emit_v_tile(j)
                    if kj + VL < NT:
                        emit_v_tile(kj + VL)
                # steady lag 2/5; once strips end, drain without idle lag
                if CFG.get("v_late", False):
                    if kj == 0:
                        for j in range(VL):
                            emit_v_tile(j)
                    if kj + VL < NT:
                        emit_v_tile(kj + VL)
                FL = CFG.get("fin_lag", 1)
                if kj < NT:
                    if 2 <= kj < NT - 1:
                        stage_att_pu(kj - 2)
                        stage_att_norm(kj - 2)
                    elif kj == NT - 1:
                        for tt in (kj - 2, kj - 1, kj):
                            stage_att_pu(tt)
                            stage_att_norm(tt)
                        for tt in range(NT - 3 - FL, NT):
                            stage_att_fin(tt)
                PL = CFG["proj_lag"]
                sched = {
                    "A": {NT: list(range(NT - PL, NT))},
                    "B": {NT - 1: [NT - PL],
                          NT: list(range(NT - PL + 1, NT))},
                    "C": {NT - 2: [NT - PL], NT - 1: [NT - PL + 1],
                          NT: list(range(NT - PL + 2, NT))},
                }[CFG.get("tail_sched", "A")]
                if PL <= kj < NT and kj - PL in sched.get(kj, []):
                    raise AssertionError("dup proj")
                if PL <= kj < NT:
                    stage_proj(kj - PL)
                if 2 + FL <= kj < NT - 1:
                    stage_att_fin(kj - 2 - FL)
                for tt in sched.get(kj, []):
                    stage_proj(tt)
                # qk chunks go last: their drains must sit BEHIND rec/ao in
                # the DVE queue, else the long wait on PE's qk matmuls
                # head-of-line blocks the per-tile normalize chain
                if CFG.get("qk_spread", "single12") == "single12":
                    if 1 <= kj <= 12:
                        i = (kj - 1) // 2
                        emit_qk_chunk(slice(512 + i * 256, 768 + i * 256),
                                      tensors=((kj - 1) % 2,))
                else:
                    if kj in (1, 2, 4, 5, 8, 9):
                        i = (1, 2, 4, 5, 8, 9).index(kj)
                        emit_qk_chunk(slice(512 + i * 256, 768 + i * 256))

    nc.compile()
    return nc


def _get_nc():
    global _CACHED_NC
    if _CACHED_NC is None:
        _CACHED_NC = _build_nc()
    return _CACHED_NC


def _to_pm(a):
    """[E, X] -> partition-major [128, KO, X] (contiguous)."""
    return np.ascontiguousarray(
        a.reshape(KO, 128, a.shape[1]).transpose(1, 0, 2))


def kernel(x, Wq, bq, Wk, bk, Wv, bv, Wp, bp):
    nc = _get_nc()
    x = np.asarray(x, np.float32)
    Wq = np.asarray(Wq, np.float32)
    Wk = np.asarray(Wk, np.float32)
    Wv = np.asarray(Wv, np.float32)
    Wp = np.asarray(Wp, np.float32)
    bq = np.asarray(bq, np.float32)
    bk = np.asarray(bk, np.float32)
    bv = np.asarray(bv, np.float32)
    bp = np.asarray(bp, np.float32)

    # d-split DR layout: column j = i*128 + 32h + p  <->  feature
    # h*64 + i*32 + p
    j = np.arange(SL)
    f = (j % 128) // 32 * 64 + (j // 128) * 32 + (j % 32)

    xs = []
    for b in range(B):
        xT = np.ascontiguousarray(x[b].T)
        x8 = _fp8(xT)
        xr = _fp8(xT - x8.astype(np.float32))
        xs.append((_to_pm(x8), _to_pm(xr)))

    in_maps = []
    for c in range(8):
        b, gq = c // 4, c % 4
        sl = slice(SL * gq, SL * (gq + 1))
        wq_s = (WS * Wq[sl][f]).astype(np.float32)
        wk_s = (WS * Wk[sl][f]).astype(np.float32)
        bq_s = (WS * bq[sl][f]).astype(np.float32)
        bk_s = (WS * bk[sl][f]).astype(np.float32)
        aux = np.zeros((128, 4), np.float32)
        aux[:, 0] = bq_s[:128]
        aux[:, 1] = bq_s[128:]
        aux[:, 2] = bk_s[:128]
        aux[:, 3] = bk_s[128:]
        wvT = np.ascontiguousarray(Wv[sl].T)           # [E, SL]
        wv8 = _fp8(VS * wvT)                # stores 4096*wv
        wvr = _fp8((wvT - wv8.astype(np.float32) / VS) * VS)
        in_maps.append({
            "x8": xs[b][0],
            "x8f": np.ascontiguousarray(xs[b][0][:, :, 0:256]),
            "xr": xs[b][1],
            "wq8": _to_pm(_fp8(np.ascontiguousarray(wq_s.T))),
            "wk8": _to_pm(_fp8(np.ascontiguousarray(wk_s.T))),
            "wv8": _to_pm(wv8),
            "wvr": _to_pm(wvr),
            "wpb": np.ascontiguousarray(Wp[:, sl].T).astype(NP_BF16),
            "aux": aux,
        })
    res = run_bass_kernel_spmd(nc, in_maps, core_ids=list(range(8)))
    ys = [np.asarray(res.results[c]["y"], np.float32) for c in range(8)]
    if any(not np.isfinite(y).all() for y in ys):
        # transient device flake observed once in ~15 runs; retry once
        res = run_bass_kernel_spmd(nc, in_maps, core_ids=list(range(8)))
        ys = [np.asarray(res.results[c]["y"], np.float32) for c in range(8)]
    # output bias: bp plus the folded v-bias contribution bv @ Wp^T
    # (exact because softmax rows sum to 1)
    ybias = bp + bv @ Wp.T
    y = np.stack([
        ys[0] + ys[1] + ys[2] + ys[3],
        ys[4] + ys[5] + ys[6] + ys[7],
    ]).astype(np.float32) + ybias[None, None, :]
    return y.astype(np.float32)


# revision 67
# speedup vs baseline: 1.0324x; 1.0028x over previous
"""Local (banded) attention kernel for Trainium2, sharded over 8 NeuronCores.

Sharding: core c handles batch b=c//4 and heads 4*(c%4)..4*(c%4)+3.
Host pre-transposes x and weight slices; device does QKV projection,
banded attention (window 128 -> only tile-diagonal +/-1 blocks), and the
per-core slice of the output projection. Host sums the 4 partial outputs
per batch and adds the output bias (including the folded V bias: since
softmax rows sum to 1, att@(v+bv) = att@v + bv, so bv@Wp^T moves to the
host-side output bias).

Mixed precision:
- q/k projections: fp8 DoubleRow with weights pre-scaled by 32 (keeps the
  small weights out of fp8's subnormal range); the energy exp scale
  absorbs the 32*32 factor.
- v projection: fp8 DoubleRow with residual compensation at one common
  scale VS=2048: v*VS = x8@wv8 + xr@wv8 + x8@wvr where x8 = fp8(x),
  xr = fp8(x - x8), wv8 = fp8(VS*wv), wvr = fp8(VS*(wv - wv8/VS)).  All
  three terms accumulate into one psum group; the VS factor cancels
  exactly because the softmax-denominator column of vaug is memset to VS
  instead of 1.  More accurate than bf16 (residual compensation), 25%
  fewer PE cycles, and replaces the 4MB bf16 x DMA with a 2MB fp8 one.
- energy / AV / output projection run in bf16.

Scheduling (found via TimelineSim-driven search): per k-tile iteration
the PE emits strips -> v(kj+1) -> att_pu(kj-2)+norm -> proj(kj-5) ->
att_fin(kj-3, transposes) -> one 256-col q-or-k projection chunk.  The
qk-chunk drains sit last so their long wait on PE's qk matmuls cannot
head-of-line-block the per-tile normalize chain in the DVE queue.
Engine split: exps + y fc0 drains on Act; rec/ao, v/qk/y fc1 drains and
attT copies on DVE; band masks (batched across the 4 heads with a
multi-dim affine_select pattern) on Pool; all DMA issue on SP.  pu and
the two bitcast bf16 transpose slots share double-buffered PSUM banks
(mm 3 + pe 3 + ut 2 = 8).

Gotcha: bass float8e4 is IEEE e4m3 (max finite 240, exponent-15 encodes
inf/nan), NOT e4m3fn -- clip to +-240 before casting on the host.
"""

import ml_dtypes
import numpy as np

import concourse.bass as bass
import concourse.mybir as mybir
from concourse import bacc
from concourse.tile import TileContext
from concourse.bass_utils import run_bass_kernel_spmd
from concourse.masks import make_identity

B, N, E, H, DH, WIN = 2, 2048, 1024, 16, 64, 128
HPC = 4              # heads per core
SL = HPC * DH        # feature slice per core (256)
NT = N // 128        # 16 query/key tiles
F32 = mybir.dt.float32
BF16 = mybir.dt.bfloat16
FP8 = mybir.dt.float8e4
KO = E // 128        # 8 contraction tiles
WS = 32.0            # q/k weight pre-scale (keeps fp8 out of subnormals)
VS = 2048.0          # v weight/psum common scale; cancelled by the 2048
                     # softmax-denominator column in vaug
SCALE_E = (1.0 / 32.0) / (WS * WS)   # exp scale: 1/sqrt(E) / (32q * 32k)
# bass float8e4 is IEEE e4m3 (max finite 240, exponent-15 encodes inf/nan)
# -- NOT e4m3fn.  Clip before casting so tails don't become inf.
NP_FP8 = ml_dtypes.float8_e4m3
NP_BF16 = ml_dtypes.bfloat16


def _fp8(a):
    return np.clip(a, -240.0, 240.0).astype(NP_FP8)

_CACHED_NC = None

# build-time experiment knobs
CFG = {
    "strip_bufs": 6,
    "io_bufs": 6,
    "small_bufs": 6,
    "ps_e_bufs": 2,
    "ps_mm_bufs": 2,
    "exp_pair": False,
    "warmups": 12,
    "y_pair": True,
    "y_mode": "act_fc0",
    "attT_g0_act": False,
    "v_lead": 1,
    "qk_spread": "single12",
    "dma_order": "v1",
    "tail_y_split": True,
    "proj_lag": 5,
    "fin_lag": 1,
    "attT_act_from": 14,
    "mask_split_from": 11,
    "mask_split_until": 4,
    "tail_y_mode": "single",
    "tail_y_split_from": 12,
    # engine split knobs
    "qk_eng": ("dve", "dve", "act", "act", "act"),  # per qk chunk emission
    "y_fc_eng": ("dve", "dve"),
}


def _build_nc():
    nc = bacc.Bacc("TRN2", target_bir_lowering=False)
    DR = mybir.MatmulPerfMode.DoubleRow

    x8_d = nc.dram_tensor("x8", [128, KO, N], FP8, kind="ExternalInput")
    x8f_d = nc.dram_tensor("x8f", [128, KO, 256], FP8, kind="ExternalInput")
    xr_d = nc.dram_tensor("xr", [128, KO, N], FP8, kind="ExternalInput")
    wq_d = nc.dram_tensor("wq8", [128, KO, SL], FP8, kind="ExternalInput")
    wk_d = nc.dram_tensor("wk8", [128, KO, SL], FP8, kind="ExternalInput")
    wv8_d = nc.dram_tensor("wv8", [128, KO, SL], FP8, kind="ExternalInput")
    wvr_d = nc.dram_tensor("wvr", [128, KO, SL], FP8, kind="ExternalInput")
    wp_d = nc.dram_tensor("wpb", [SL, E], BF16, kind="ExternalInput")
    if CFG.get("y_fp8", False) or CFG.get("y_fp8x", False):
        wph_d = nc.dram_tensor("wph", [SL, E], FP8, kind="ExternalInput")
        wpr_d = nc.dram_tensor("wpr", [SL, E], FP8, kind="ExternalInput")
    aux_d = nc.dram_tensor("aux", [128, 4], F32, kind="ExternalInput")
    y_d = nc.dram_tensor("y", [N, E], BF16, kind="ExternalOutput")

    with TileContext(nc) as tc:
        with (
            tc.tile_pool(name="const", bufs=1) as const,
            tc.tile_pool(name="persist", bufs=1) as persist,
            tc.tile_pool(name="io", bufs=CFG["io_bufs"]) as io,
            tc.tile_pool(name="small", bufs=CFG["small_bufs"]) as small,
            tc.tile_pool(name="strips", bufs=CFG["strip_bufs"]) as strip_pool,
            tc.tile_pool(name="ps_mm", bufs=(3 if not CFG.get("exp_pair", True) else CFG["ps_mm_bufs"]), space="PSUM") as ps_mm,
            tc.tile_pool(name="ps_e", bufs=(3 if not CFG.get("exp_pair", True) else CFG["ps_e_bufs"]), space="PSUM") as ps_e,
            tc.tile_pool(name="ps_ut", bufs=2, space="PSUM") as ps_ut,
        ):
            # ---- persistent SBUF tensors ----
            x8_sb = persist.tile([128, KO, N], FP8)
            x8f_sb = persist.tile([128, KO, 256], FP8)
            xr_sb = persist.tile([128, KO, N], FP8)
            wq_sb = persist.tile([128, KO, SL], FP8)
            wk_sb = persist.tile([128, KO, SL], FP8)
            wv8_sb = persist.tile([128, KO, SL], FP8)
            wvr_sb = persist.tile([128, KO, SL], FP8)
            if CFG.get("y_fp8", False) or CFG.get("y_fp8x", False):
                wph_sb = persist.tile([128, 2, E], FP8)
                wpr_sb = persist.tile([128, 2, E], FP8)
            else:
                wp_sb = persist.tile([128, 2, E], BF16)
            aux = const.tile([128, 4], F32)

            def xc(sb, d, c0, c1, eng=None):  # column chunk of x8/xr
                (eng or nc.sync).dma_start(sb[:, :, c0:c1], d.ap()[:, :, c0:c1])

            # DMA order: feed q/k proj first, then v inputs, wp before
            # stage_proj(0) fires, rest of x by strip consumption order.
            if CFG.get("dma_order", "v2") in ("v4", "v5"):
                # first x8 chunk(s) via Pool SWDGE: lower launch latency and
                # a second issue queue for the prologue-critical bytes
                xc(x8_sb, x8_d, 0, 512, eng=nc.gpsimd)
                nc.sync.dma_start(wq_sb[:], wq_d.ap())
                nc.sync.dma_start(aux[:], aux_d.ap())
                nc.sync.dma_start(wk_sb[:], wk_d.ap())
                if CFG["dma_order"] == "v4":
                    xc(x8_sb, x8_d, 512, 1024, eng=nc.gpsimd)
                else:
                    xc(x8_sb, x8_d, 512, 1024)
            elif CFG.get("dma_order", "v2") == "v3":
                nc.sync.dma_start(wq_sb[:], wq_d.ap())
                xc(x8_sb, x8_d, 0, 128)
                nc.sync.dma_start(aux[:], aux_d.ap())
                nc.sync.dma_start(wk_sb[:], wk_d.ap())
                xc(x8_sb, x8_d, 128, 512)
                xc(x8_sb, x8_d, 512, 1024)
            elif CFG.get("dma_order", "v2") == "v2":
                xc(x8_sb, x8_d, 0, 128)
                nc.sync.dma_start(wq_sb[:], wq_d.ap())
                nc.sync.dma_start(aux[:], aux_d.ap())
                nc.sync.dma_start(wk_sb[:], wk_d.ap())
                xc(x8_sb, x8_d, 128, 640)
                xc(x8_sb, x8_d, 640, 1024)
            elif CFG.get("x8_fast", False):
                nc.sync.dma_start(wq_sb[:], wq_d.ap())
                nc.sync.dma_start(x8f_sb[:], x8f_d.ap())
                nc.sync.dma_start(aux[:], aux_d.ap())
                nc.sync.dma_start(wk_sb[:], wk_d.ap())
                xc(x8_sb, x8_d, 0, 512)
                xc(x8_sb, x8_d, 512, 1024)
            else:
                nc.sync.dma_start(wq_sb[:], wq_d.ap())
                xc(x8_sb, x8_d, 0, 512)
                nc.sync.dma_start(aux[:], aux_d.ap())
                nc.sync.dma_start(wk_sb[:], wk_d.ap())
                xc(x8_sb, x8_d, 512, 1024)
            nc.sync.dma_start(wv8_sb[:], wv8_d.ap())
            nc.sync.dma_start(wvr_sb[:], wvr_d.ap())
            xc(xr_sb, xr_d, 0, 512)
            xc(x8_sb, x8_d, 1024, 1536)
            xc(xr_sb, xr_d, 512, 1024)
            if CFG.get("y_fp8", False) or CFG.get("y_fp8x", False):
                nc.sync.dma_start(
                    wph_sb[:], wph_d.ap().rearrange("(g p) f -> p g f", p=128))
                nc.sync.dma_start(
                    wpr_sb[:], wpr_d.ap().rearrange("(g p) f -> p g f", p=128))
            else:
                nc.sync.dma_start(
                    wp_sb[:], wp_d.ap().rearrange("(g p) f -> p g f", p=128))
            xc(x8_sb, x8_d, 1536, 2048)
            xc(xr_sb, xr_d, 1024, 1536)
            xc(xr_sb, xr_d, 1536, 2048)

            bq_col = aux[:, 0:2]
            bk_col = aux[:, 2:4]

            # ---- on-chip constants ----
            warm = const.tile([128, 128], BF16)
            nc.gpsimd.memset(warm[:], 0.0)
            ident = const.tile([128, 128], BF16)
            make_identity(nc, ident[:])

            # ---- projection outputs ----
            qsb = persist.tile([128, 2, N], FP8, name="qsb")
            ksb = persist.tile([128, 2, N], FP8, name="ksb")
            vaug = persist.tile([128, NT, HPC, DH + 1], BF16)
            nc.gpsimd.memset(vaug[:, :, :, DH], float(VS))
            attT = persist.tile([128, 2, N], BF16, name="attT")
            if CFG.get("y_fp8", False) or CFG.get("y_fp8x", False):
                a16 = persist.tile([128, 2, N], FP8, name="a16")
                ar8 = persist.tile([128, 2, N], FP8, name="ar8")

            qk_emit_idx = [0]

            def emit_qk_chunk(cs, tensors=(0, 1), pre=False, xf=None):
                w_cs = cs.stop - cs.start
                for ti, (w_sb, out_t, b_col) in enumerate(
                        ((wq_sb, qsb, bq_col), (wk_sb, ksb, bk_col))):
                    if ti not in tensors:
                        continue
                    if CFG.get("qk_merge_g", True) and w_cs <= 256:
                        # both g halves in one psum bank -> one drain
                        ps = ps_mm.tile([128, 2, 256], F32, tag="mm",
                                        name="ps_qk")
                        xs_src = xf if xf is not None else x8_sb
                        for g in range(2):
                            for kp in range(KO // 2):
                                nc.tensor.matmul(
                                    ps[:, g, :w_cs],
                                    lhsT=w_sb[:, 2 * kp:2 * kp + 2,
                                              g * 128:(g + 1) * 128],
                                    rhs=xs_src[:, 2 * kp:2 * kp + 2, cs],
                                    start=(kp == 0),
                                    stop=(kp == KO // 2 - 1),
                                    perf_mode=DR)
                        if ti == 1 and CFG.get("qk_pre_mix", False) and pre:
                            for g in range(2):
                                nc.scalar.activation(
                                    out_t[:, g, cs], ps[:, g, :w_cs],
                                    mybir.ActivationFunctionType.Identity,
                                    bias=b_col[:, g:g + 1])
                        else:
                            nc.vector.tensor_tensor(
                                out_t[:, :, cs], ps[:, :, :w_cs],
                                b_col[:, :, None].broadcast_to([128, 2, w_cs]),
                                mybir.AluOpType.add)
                        continue
                    for g in range(2):
                        ps = ps_mm.tile([128, 512], F32, tag="mm", name="ps_qk")
                        ps = ps[:, :w_cs]
                        for kp in range(KO // 2):
                            nc.tensor.matmul(
                                ps,
                                lhsT=w_sb[:, 2 * kp:2 * kp + 2, g * 128:(g + 1) * 128],
                                rhs=x8_sb[:, 2 * kp:2 * kp + 2, cs],
                                start=(kp == 0), stop=(kp == KO // 2 - 1),
                                perf_mode=DR)
                        if CFG.get("qk_mix", False) and ti == 0:
                            nc.scalar.activation(
                                out_t[:, g, cs], ps,
                                mybir.ActivationFunctionType.Identity,
                                bias=b_col[:, g:g + 1])
                        else:
                            nc.vector.tensor_scalar_add(
                                out_t[:, g, cs], ps, b_col[:, g:g + 1])

            def emit_v_tile(nt):
                ps = ps_mm.tile([128, 512], F32, tag="mm", name="ps_v")
                psv = ps[:, :SL]
                rs = slice(nt * 128, (nt + 1) * 128)
                terms = ((x8_sb, wv8_sb), (xr_sb, wv8_sb), (x8_sb, wvr_sb))
                for ti, (xs_, ws_) in enumerate(terms):
                    for k2 in range(KO // 2):
                        nc.tensor.matmul(
                            psv, lhsT=xs_[:, 2 * k2:2 * k2 + 2, rs],
                            rhs=ws_[:, 2 * k2:2 * k2 + 2, :],
                            start=(ti == 0 and k2 == 0),
                            stop=(ti == 2 and k2 == KO // 2 - 1),
                            perf_mode=DR)
                if CFG.get("v_alt", False) and nt % 2 == 0:
                    nc.scalar.activation(
                        vaug[:, nt, :, :DH],
                        psv.rearrange("p (h d) -> p h d", d=DH),
                        mybir.ActivationFunctionType.Copy)
                else:
                    nc.vector.tensor_copy(
                        vaug[:, nt, :, :DH],
                        psv.rearrange("p (h d) -> p h d", d=DH))

            # ---- banded attention ----
            strips = {}

            def emit_strips(kj):
                lo, hi = max(0, kj - 1), min(NT - 1, kj + 1)
                w = (hi - lo + 1) * 128
                st4 = strip_pool.tile([128, HPC, 384], BF16, tag="strip",
                                      name="st4")
                if CFG.get("exp_pair", True):
                    for hp in range(HPC // 2):
                        # two heads share a 2-bank psum tile so the exp
                        # drains as one wide Activation instruction
                        pe = ps_e.tile([128, 2, 512], F32, tag="pe", name="pe")
                        for hh in range(2):
                            h = 2 * hp + hh
                            hb = 32 * h
                            nc.tensor.matmul(
                                pe[:, hh, :w],
                                lhsT=ksb[hb:hb + 32, :, kj * 128:(kj + 1) * 128],
                                rhs=qsb[hb:hb + 32, :, lo * 128:(hi + 1) * 128],
                                start=True, stop=True, perf_mode=DR,
                                tile_position=(hb, 0))
                        nc.scalar.activation(
                            st4[:, 2 * hp:2 * hp + 2, :w], pe[:, :, :w],
                            mybir.ActivationFunctionType.Exp, scale=SCALE_E)
                else:
                    split = (kj >= CFG.get("mask_split_from", 99)
                             or kj < CFG.get("mask_split_until", 0))
                    for h in range(HPC):
                        pe = ps_e.tile([128, 384], F32, tag="pe", name="pe")
                        hb = 32 * h
                        nc.tensor.matmul(
                            pe[:, :w],
                            lhsT=ksb[hb:hb + 32, :, kj * 128:(kj + 1) * 128],
                            rhs=qsb[hb:hb + 32, :, lo * 128:(hi + 1) * 128],
                            start=True, stop=True, perf_mode=DR,
                            tile_position=(hb, 0))
                        nc.scalar.activation(
                            st4[:, h, :w], pe[:, :w],
                            mybir.ActivationFunctionType.Exp, scale=SCALE_E)
                        if split and h % 2 == 1:
                            emit_masks(st4, kj, lo, hi, slice(h - 1, h + 1), 2)
                if (kj < CFG.get("mask_split_from", 99)
                        and kj >= CFG.get("mask_split_until", 0)):
                    emit_masks(st4, kj, lo, hi, slice(0, HPC), HPC)
                strips[kj] = (st4, lo)

            def emit_masks(st4, kj, lo, hi, hs, nh):
                # band masks, batched across heads (Pool, SBUF-only).
                # U block (q-tile kj-1): keep c >= p; L block: keep c <= p.
                if lo == kj - 1:
                    nc.gpsimd.affine_select(
                        out=st4[:, hs, 0:128], in_=st4[:, hs, 0:128],
                        compare_op=mybir.AluOpType.is_ge, fill=0.0, base=0,
                        pattern=[[0, nh], [1, 128]], channel_multiplier=-1)
                if hi == kj + 1:
                    lc = (hi - lo) * 128
                    nc.gpsimd.affine_select(
                        out=st4[:, hs, lc:lc + 128], in_=st4[:, hs, lc:lc + 128],
                        compare_op=mybir.AluOpType.is_ge, fill=0.0, base=0,
                        pattern=[[0, nh], [-1, 128]], channel_multiplier=1)

            att_state = {}

            def stage_att(t):
                stage_att_pu(t)
                stage_att_norm(t)
                stage_att_fin(t)

            def stage_att_pu(t):
                ks = [k for k in (t - 1, t, t + 1) if 0 <= k < NT]
                # pu (4*65 f32) plus two bitcast bf16 transpose slots share
                # each 1-bank ut tile; bufs=2 so tile t+1 never waits on
                # tile t's attT copies
                ut = ps_ut.tile([128, 512], F32, tag="ut", name="ut")
                pu = ut[:, 0:HPC * (DH + 1)].rearrange(
                    "p (h d) -> p h d", d=DH + 1)
                for h in range(HPC):
                    for i, k2 in enumerate(ks):
                        st4, lo2 = strips[k2]
                        col = (t - lo2) * 128
                        nc.tensor.matmul(
                            pu[:, h, :], lhsT=st4[:, h, col:col + 128],
                            rhs=vaug[:, k2, h, :],
                            start=(i == 0),
                            stop=(i == len(ks) - 1),
                            skip_group_check=True)
                att_state[t] = [ut, pu, None]

            def stage_att_norm(t):
                ut, pu, _ = att_state[t]
                ao = small.tile([128, HPC, DH], BF16, tag="ao", name="ao")
                if CFG.get("ao_div", False):
                    nc.vector.tensor_tensor(
                        ao[:], pu[:, :, :DH],
                        pu[:, :, DH:DH + 1].broadcast_to([128, HPC, DH]),
                        mybir.AluOpType.divide)
                else:
                    rec = small.tile([128, HPC], F32, tag="rec", name="rec")
                    nc.vector.reciprocal(rec[:], pu[:, :, DH])
                    nc.vector.tensor_mul(
                        ao[:], pu[:, :, :DH],
                        rec[:, :, None].broadcast_to([128, HPC, DH]))
                att_state[t][2] = ao

            def stage_att_fin(t):
                ts_ = slice(t * 128, (t + 1) * 128)
                ut, pu, ao = att_state.pop(t)
                if CFG.get("y_fp8x", False):
                    # SBUF->SBUF XBAR transpose; Pool then derives the fp8
                    # main+residual pair from attT without touching PSUM
                    nc.sync.dma_start_transpose(attT[:, :, ts_], ao[:])
                    nc.gpsimd.tensor_scalar_mul(
                        a16[:, :, ts_], attT[:, :, ts_], 16.0)
                    nc.gpsimd.scalar_tensor_tensor(
                        out=ar8[:, :, ts_], in0=attT[:, :, ts_],
                        scalar=16.0, in1=a16[:, :, ts_],
                        op0=mybir.AluOpType.mult,
                        op1=mybir.AluOpType.subtract)
                    return
                for g in range(2):
                    pt = ut[:, 320 + 64 * g:384 + 64 * g].bitcast(BF16)
                    nc.tensor.transpose(
                        pt, ao[:, 2 * g:2 * g + 2, :], ident[:])
                if CFG.get("attT_merge", True):
                    ptb = ut[:, 320:448].bitcast(BF16)
                    if (t >= CFG.get("attT_act_from", 99)
                            or t < CFG.get("attT_act_until", 0)):
                        nc.scalar.activation(
                            attT[:, :, ts_],
                            ptb.rearrange("p (g q) -> p g q", g=2),
                            mybir.ActivationFunctionType.Copy)
                    else:
                        nc.vector.tensor_copy(
                            attT[:, :, ts_],
                            ptb.rearrange("p (g q) -> p g q", g=2))
                    if CFG.get("y_fp8", False):
                        nc.gpsimd.tensor_scalar_mul(
                            a16[:, :, ts_], attT[:, :, ts_], 16.0)
                        nc.gpsimd.scalar_tensor_tensor(
                            out=ar8[:, :, ts_], in0=attT[:, :, ts_],
                            scalar=16.0, in1=a16[:, :, ts_],
                            op0=mybir.AluOpType.mult,
                            op1=mybir.AluOpType.subtract)
                else:
                    for g in range(2):
                        pt = ut[:, 320 + 64 * g:384 + 64 * g].bitcast(BF16)
                        if g == 0 and CFG.get("attT_g0_act", False):
                            nc.scalar.activation(
                                attT[:, g, ts_], pt,
                                mybir.ActivationFunctionType.Copy)
                        else:
                            nc.vector.tensor_copy(attT[:, g, ts_], pt)

            ybuf = {}

            def stage_proj(t):
                ts_ = slice(t * 128, (t + 1) * 128)
                tail = (t >= CFG.get("tail_y_split_from", NT - 2)
                        and CFG.get("tail_y_split", True))
                if CFG["y_pair"] and not tail:
                    if t % 2 == 0:
                        ybuf["t"] = io.tile([128, 2, E], BF16, tag="y2",
                                            name="y2_sb")
                    y_sb = ybuf["t"][:, t % 2, :]
                else:
                    y_sb = io.tile([128, E], BF16, tag="y", name="y_sb")
                for fc in range(2):
                    ps = ps_mm.tile([128, 512], F32, tag="mm", name="ps_yt")
                    fs = slice(fc * 512, (fc + 1) * 512)
                    if CFG.get("y_fp8", False) or CFG.get("y_fp8x", False):
                        terms = ((a16, wph_sb), (ar8, wph_sb), (a16, wpr_sb))
                        for ti2, (a_, w_) in enumerate(terms):
                            nc.tensor.matmul(
                                ps[:], lhsT=a_[:, :, ts_],
                                rhs=w_[:, :, fs],
                                start=(ti2 == 0), stop=(ti2 == 2),
                                perf_mode=DR)
                    else:
                        for g in range(2):
                            nc.tensor.matmul(
                                ps[:],
                                lhsT=attT[:, g, ts_],
                                rhs=wp_sb[:, g, fs],
                                start=(g == 0), stop=(g == 1))
                    if t >= CFG.get("tail_drain_split_from", 99):
                        h2 = slice(fc * 512, fc * 512 + 256)
                        h3 = slice(fc * 512 + 256, (fc + 1) * 512)
                        nc.scalar.activation(
                            y_sb[:, h2], ps[:, 0:256],
                            mybir.ActivationFunctionType.Copy)
                        nc.vector.tensor_copy(y_sb[:, h3], ps[:, 256:512])
                        if tail and CFG.get("tail_y_mode", "fc") == "fc":
                            nc.sync.dma_start(y_d[ts_, fs], y_sb[:, fs])
                        continue
                    ym = CFG.get("y_mode", "alt")
                    on_act = {"alt": (t + fc) % 2 == 0,
                              "dve": False,
                              "act_fc0": fc == 0,
                              "quarter": (t % 2 == 0) and fc == 0}[ym]
                    if t >= CFG.get("y_both_act_from", 99):
                        on_act = True
                    ysc = (1.0 / 2048.0
                           if CFG.get("y_fp8", False) or CFG.get("y_fp8x", False)
                           else None)
                    if on_act:
                        if ysc is None:
                            nc.scalar.activation(
                                y_sb[:, fs], ps[:],
                                mybir.ActivationFunctionType.Copy)
                        else:
                            nc.scalar.activation(
                                y_sb[:, fs], ps[:],
                                mybir.ActivationFunctionType.Identity,
                                scale=ysc)
                    elif ysc is None:
                        nc.vector.tensor_copy(y_sb[:, fs], ps[:])
                    else:
                        nc.vector.tensor_scalar_mul(y_sb[:, fs], ps[:], ysc)
                    if tail and CFG.get("tail_y_mode", "fc") == "fc":
                        # drain latency off the critical tail: ship each
                        # 512-col half as soon as its copy lands
                        nc.sync.dma_start(y_d[ts_, fs], y_sb[:, fs])
                if tail:
                    if CFG.get("tail_y_mode", "fc") == "single":
                        nc.sync.dma_start(y_d[ts_, :], y_sb[:])
                elif CFG["y_pair"]:
                    if t % 2 == 1:
                        dst = y_d[(t - 1) * 128:(t + 1) * 128, :]
                        nc.sync.dma_start(
                            dst.rearrange("(tt p) f -> p tt f", p=128),
                            ybuf["t"][:])
                else:
                    nc.sync.dma_start(y_d[ts_, :], y_sb[:])

            # ---- schedule ----
            # PE warmup against the p-state ramp while input DMAs stream
            for i in range(CFG["warmups"]):
                if CFG.get("exp_pair", True):
                    pw = ps_e.tile([128, 2, 512], F32, tag="pe", name="pe_w")
                    pw = pw[:, 0, :128]
                else:
                    pw = ps_e.tile([128, 384], F32, tag="pe", name="pe_w")
                    pw = pw[:, :128]
                nc.tensor.matmul(pw, lhsT=warm[:], rhs=warm[:],
                                 start=True, stop=True)
            if CFG.get("qk_pre_merge", True):
                emit_qk_chunk(slice(0, 256), pre=True,
                              xf=(x8f_sb if CFG.get("x8_fast", False) else None))
                emit_qk_chunk(slice(256, 512), pre=True)
            else:
                emit_qk_chunk(slice(0, 128))
                emit_qk_chunk(slice(128, 512))
            VL = CFG["v_lead"]
            for kj in range(NT + 1):
                if kj < NT:
                    emit_strips(kj)
                if not CFG.get("v_late", False):
                    if kj == 0:
                        for j in range(VL):
                            emit_v_tile(j)
                    if kj + VL < NT:
                        emit_v_tile(kj + VL)
                # steady lag 2/5; once strips end, drain without idle lag
                if CFG.get("v_late", False):
                    if kj == 0:
                        for j in range(VL):
                            emit_v_tile(j)
                    if kj + VL < NT:
                        emit_v_tile(kj + VL)
                FL = CFG.get("fin_lag", 1)
                if kj < NT:
                    if 2 <= kj < NT - 1:
                        stage_att_pu(kj - 2)
                        stage_att_norm(kj - 2)
                    elif kj == NT - 1:
                        for tt in (kj - 2, kj - 1, kj):
                            stage_att_pu(tt)
                            stage_att_norm(tt)
                        for tt in range(NT - 3 - FL, NT):
                            stage_att_fin(tt)
                PL = CFG["proj_lag"]
                sched = {
                    "A": {NT: list(range(NT - PL, NT))},
                    "B": {NT - 1: [NT - PL],
                          NT: list(range(NT - PL + 1, NT))},
                    "C": {NT - 2: [NT - PL], NT - 1: [NT - PL + 1],
                          NT: list(range(NT - PL + 2, NT))},
                }[CFG.get("tail_sched", "A")]
                if PL <= kj < NT and kj - PL in sched.get(kj, []):
                    raise AssertionError("dup proj")
                if PL <= kj < NT:
                    stage_proj(kj - PL)
                if 2 + FL <= kj < NT - 1:
                    stage_att_fin(kj - 2 - FL)
                for tt in sched.get(kj, []):
                    stage_proj(tt)
                # qk chunks go last: their drains must sit BEHIND rec/ao in
                # the DVE queue, else the long wait on PE's qk matmuls
                # head-of-line blocks the per-tile normalize chain
                if CFG.get("qk_spread", "single12") == "single12":
                    if 1 <= kj <= 12:
                        i = (kj - 1) // 2
                        emit_qk_chunk(slice(512 + i * 256, 768 + i * 256),
                                      tensors=((kj - 1) % 2,))
                else:
                    if kj in (1, 2, 4, 5, 8, 9):
                        i = (1, 2, 4, 5, 8, 9).index(kj)
                        emit_qk_chunk(slice(512 + i * 256, 768 + i * 256))

    nc.compile()
    return nc


def _get_nc():
    global _CACHED_NC
    if _CACHED_NC is None:
        _CACHED_NC = _build_nc()
    return _CACHED_NC


def _to_pm(a):
    """[E, X] -> partition-major [128, KO, X] (contiguous)."""
    return np.ascontiguousarray(
        a.reshape(KO, 128, a.shape[1]).transpose(1, 0, 2))


def kernel(x, Wq, bq, Wk, bk, Wv, bv, Wp, bp):
    nc = _get_nc()
    x = np.asarray(x, np.float32)
    Wq = np.asarray(Wq, np.float32)
    Wk = np.asarray(Wk, np.float32)
    Wv = np.asarray(Wv, np.float32)
    Wp = np.asarray(Wp, np.float32)
    bq = np.asarray(bq, np.float32)
    bk = np.asarray(bk, np.float32)
    bv = np.asarray(bv, np.float32)
    bp = np.asarray(bp, np.float32)

    # d-split DR layout: column j = i*128 + 32h + p  <->  feature
    # h*64 + i*32 + p
    j = np.arange(SL)
    f = (j % 128) // 32 * 64 + (j // 128) * 32 + (j % 32)

    xs = []
    for b in range(B):
        xT = np.ascontiguousarray(x[b].T)
        x8 = _fp8(xT)
        xr = _fp8(xT - x8.astype(np.float32))
        xs.append((_to_pm(x8), _to_pm(xr)))

    in_maps = []
    for c in range(8):
        b, gq = c // 4, c % 4
        sl = slice(SL * gq, SL * (gq + 1))
        wq_s = (WS * Wq[sl][f]).astype(np.float32)
        wk_s = (WS * Wk[sl][f]).astype(np.float32)
        bq_s = (WS * bq[sl][f]).astype(np.float32)
        bk_s = (WS * bk[sl][f]).astype(np.float32)
        aux = np.zeros((128, 4), np.float32)
        aux[:, 0] = bq_s[:128]
        aux[:, 1] = bq_s[128:]
        aux[:, 2] = bk_s[:128]
        aux[:, 3] = bk_s[128:]
        wvT = np.ascontiguousarray(Wv[sl].T)           # [E, SL]
        wv8 = _fp8(VS * wvT)                # stores 4096*wv
        wvr = _fp8((wvT - wv8.astype(np.float32) / VS) * VS)
        in_maps.append({
            "x8": xs[b][0],
            "x8f": np.ascontiguousarray(xs[b][0][:, :, 0:256]),
            "xr": xs[b][1],
            "wq8": _to_pm(_fp8(np.ascontiguousarray(wq_s.T))),
            "wk8": _to_pm(_fp8(np.ascontiguousarray(wk_s.T))),
            "wv8": _to_pm(wv8),
            "wvr": _to_pm(wvr),
            "wpb": np.ascontiguousarray(Wp[:, sl].T).astype(NP_BF16),
            "aux": aux,
        })
    res = run_bass_kernel_spmd(nc, in_maps, core_ids=list(range(8)))
    ys = [np.asarray(res.results[c]["y"], np.float32) for c in range(8)]
    if any(not np.isfinite(y).all() for y in ys):
        # transient device flake observed once in ~15 runs; retry once
        res = run_bass_kernel_spmd(nc, in_maps, core_ids=list(range(8)))
        ys = [np.asarray(res.results[c]["y"], np.float32) for c in range(8)]
    # output bias: bp plus the folded v-bias contribution bv @ Wp^T
    # (exact because softmax rows sum to 1)
    ybias = bp + bv @ Wp.T
    y = np.stack([
        ys[0] + ys[1] + ys[2] + ys[3],
        ys[4] + ys[5] + ys[6] + ys[7],
    ]).astype(np.float32) + ybias[None, None, :]
    return y.astype(np.float32)
